# revision 8
# baseline (speedup 1.0000x reference)
"""Llama attention layer (B=1, S=2048, H=32, KVH=8, D=128, HID=4096) on 8 TRN2
NeuronCores.

Sharding: tensor-parallel over head groups. Core c computes Q heads
[4c..4c+4) and KV head c end-to-end (QKV projection, RoPE, causal GQA
attention, o_proj rows for its heads), then a chunked ReduceScatter sums the
o_proj partials so core c ends up with rows {512j + 64c .. 512j + 64c + 64}
of the output for j in 0..3. The host reassembles the full [2048, 4096]
output by concatenating the shards.

v2 design (fp16 end-to-end, pipelined):
  - All DRAM-resident tensors are fp16 (half the HBM traffic and half the
    collective bytes of fp32; fp16 matmuls run at full PE rate and carry
    10-bit mantissas). PSUM accumulation stays fp32.
  - Single-pass QKV: the whole wqkv shard (6.3 MB fp16) is SBUF-resident,
    hT streams through once. RoPE runs on DVE per chunk, overlapped with
    the next chunk's QKV matmuls.
  - Softmax denominators accumulate on DVE (esum += exp tile) instead of
    per-tile PE ones-matmuls; one [1,512] ones-matmul per (chunk, head)
    reduces esum across partitions.
  - o_proj matmul groups of chunk c-1 are interleaved into the attention
    t-loop of chunk c so the PE never idles waiting on the scalar engine's
    exp tiles.
  - Per-chunk fp16 ReduceScatter overlaps the remaining compute.
"""

import sys

if "/opt/trn_rl_repo" not in sys.path:
    sys.path.insert(0, "/opt/trn_rl_repo")

import numpy as np

# Model dims (hardcoded per problem spec)
H, KVH, D, HID = 32, 8, 128, 4096
S = 2048
THETA = 10000.0
NCORES = 8
QH = H // NCORES          # 4 query heads per core
P = 128                   # partitions
SC = 512                  # sequence chunk (matmul free dim)
NS = S // SC              # 4 chunks
KT = HID // P             # 32 contraction tiles for the projections
ST = S // P               # 16 sequence tiles of 128
NQK = QH + 2              # col-tiles per core in wqkv: q0..q3, k, v
WCOLS = NQK * P           # 768
GK = 4                    # weight k-tiles per DMA group
NG = KT // GK             # 8 groups
ISQRT_D = float(D) ** -0.5

_CACHE = {}


def _build():
    import concourse.bass as bass
    import concourse.tile as tile
    from concourse import bacc, mybir
    from contextlib import ExitStack

    F32 = mybir.dt.float32
    F32R = mybir.dt.float32r
    F16 = mybir.dt.float16
    AF = mybir.ActivationFunctionType

    nc = bacc.Bacc(
        "TRN2",
        target_bir_lowering=False,
        debug=False,
        enable_asserts=True,
        num_devices=NCORES,
    )

    hT = nc.dram_tensor("hT", [HID, S], F16, kind="ExternalInput").ap()
    wqkv = nc.dram_tensor("wqkv", [HID, WCOLS], F16, kind="ExternalInput").ap()
    wo = nc.dram_tensor("wo", [QH * D, HID], F16, kind="ExternalInput").ap()
    cos2 = nc.dram_tensor("cos2", [P, S], F16, kind="ExternalInput").ap()
    sinn2 = nc.dram_tensor("sinn2", [P, S], F16, kind="ExternalInput").ap()
    maskd = nc.dram_tensor("maskd", [P, 4 * P], F16, kind="ExternalInput").ap()
    ident = nc.dram_tensor("ident", [P, P], F16, kind="ExternalInput").ap()
    onesd = nc.dram_tensor("onesd", [P, 1], F32, kind="ExternalInput").ap()
    out = nc.dram_tensor("out", [S // NCORES, HID], F16, kind="ExternalOutput").ap()
    partial = nc.dram_tensor("partial", [S, HID], F16).ap()
    rs_out = nc.dram_tensor("rs_out", [S // NCORES, HID], F16).ap()

    groups = [list(range(NCORES))]

    with tile.TileContext(nc) as tc:
        with ExitStack() as ctx:
            # ---------------- constants (whole-kernel lifetime) ----------------
            cpool = ctx.enter_context(tc.tile_pool(name="const", bufs=1))
            mask_t = cpool.tile([P, 4 * P], F16, name="mask_t")
            ident_t = cpool.tile([P, P], F16, name="ident_t")
            ones_t = cpool.tile([P, 1], F32R, name="ones_t")
            cos_t = cpool.tile([P, S], F16, name="cos_t")
            sin_t = cpool.tile([P, S], F16, name="sin_t")
            nc.sync.dma_start(mask_t[:], maskd)
            nc.sync.dma_start(ident_t[:], ident)
            nc.sync.dma_start(ones_t[:], onesd.bitcast(F32R))
            nc.sync.dma_start(cos_t[:], cos2)
            nc.sync.dma_start(sin_t[:], sinn2)

            # ------------- persistent activation buffers -------------
            ppool = ctx.enter_context(tc.tile_pool(name="persist", bufs=1))
            qk = {}
            for n in range(QH + 1):
                for c in range(NS):
                    qk[(n, c)] = ppool.tile(
                        [P, SC], F16, name=f"qk{n}_{c}", tag=f"qk{n}_{c}"
                    )
            vT = [
                ppool.tile([P, SC], F16, name=f"vT{c}", tag=f"vT{c}")
                for c in range(NS)
            ]
            vnat = [
                ppool.tile([P, P], F16, name=f"vn{t}", tag=f"vn{t}")
                for t in range(ST)
            ]
            attnT = {}
            for h in range(QH):
                for c in range(NS):
                    attnT[(h, c)] = ppool.tile(
                        [P, SC], F16, name=f"at{h}_{c}", tag=f"at{h}_{c}"
                    )

            # ------------- resident weights (wqkv + wo, fp16) -------------
            wpool = ctx.enter_context(tc.tile_pool(name="wres", bufs=1))
            wo_t = [
                wpool.tile([P, HID], F16, name=f"wo{hh}", tag=f"wo{hh}")
                for hh in range(QH)
            ]
            for hh in range(QH):
                nc.sync.dma_start(wo_t[hh][:], wo[hh * P : (hh + 1) * P, :])

            wq_g = {}

            def load_group(g):
                wq_g[g] = wpool.tile(
                    [P, GK * WCOLS], F16, name=f"wqg{g}", tag=f"wqg{g}"
                )
                src = wqkv[g * GK * P : (g + 1) * GK * P, :].rearrange(
                    "(t p) n -> p t n", p=P
                )
                dst = wq_g[g][:].rearrange("p (t n) -> p t n", t=GK)
                nc.sync.dma_start(dst, src)

            def wslice(k, n):
                g, kk = divmod(k, GK)
                off = kk * WCOLS + n * P
                return wq_g[g][:, off : off + P]

            for g in range(NG):
                load_group(g)

            # ---------------- stage A: QKV projection + RoPE + vT ------------
            half = P // 2
            with tc.tile_pool(name="ht", bufs=4) as h_pool, tc.tile_pool(
                name="psA", bufs=NQK, space="PSUM"
            ) as psA, tc.tile_pool(
                name="pst", bufs=2, space="PSUM"
            ) as pst, tc.tile_pool(name="ropet", bufs=3) as rpool:
                for c in range(NS):
                    ps = [
                        psA.tile([P, SC], F32, name=f"psA{n}", tag="psA")
                        for n in range(NQK)
                    ]
                    for k in range(KT):
                        ht_t = h_pool.tile([P, SC], F16, name="ht_t", tag="ht")
                        nc.sync.dma_start(
                            ht_t[:], hT[k * P : (k + 1) * P, c * SC : (c + 1) * SC]
                        )
                        for n in range(NQK):
                            nc.tensor.matmul(
                                ps[n][:],
                                wslice(k, n),
                                ht_t[:],
                                start=(k == 0),
                                stop=(k == KT - 1),
                            )
                    for n in range(QH + 1):
                        nc.scalar.copy(qk[(n, c)][:], ps[n][:])
                    nc.scalar.copy(vT[c][:], ps[NQK - 1][:])

                    # RoPE in place on this chunk's q heads and k (DVE),
                    # overlapped with the next chunk's QKV matmuls
                    csl = cos_t[:, c * SC : (c + 1) * SC]
                    ssl = sin_t[:, c * SC : (c + 1) * SC]
                    for n in range(QH + 1):
                        src = qk[(n, c)]
                        swp = rpool.tile([P, SC], F16, name="swp", tag="swp")
                        t1 = rpool.tile([P, SC], F16, name="t1", tag="t1")
                        nc.sync.dma_start(swp[0:half, :], src[half:P, :])
                        nc.sync.dma_start(swp[half:P, :], src[0:half, :])
                        nc.vector.tensor_mul(t1[:], src[:], csl)
                        nc.vector.tensor_mul(swp[:], swp[:], ssl)
                        nc.vector.tensor_add(src[:], t1[:], swp[:])

                # v transpose to natural [s, d]
                for t in range(ST):
                    c, j = divmod(t, NS)
                    tp = pst.tile([P, P], F16, name="tp", tag="tp")
                    nc.tensor.transpose(
                        tp[:], vT[c][:, j * P : (j + 1) * P], ident_t[:]
                    )
                    nc.scalar.copy(vnat[t][:], tp[:])

            # ---------- attention + o_proj + reduce-scatter ----------
            with tc.tile_pool(name="pssc", bufs=3, space="PSUM") as ps_sc, tc.tile_pool(
                name="pssm", bufs=1, space="PSUM"
            ) as ps_sm, tc.tile_pool(
                name="pspv", bufs=2, space="PSUM"
            ) as ps_pv, tc.tile_pool(
                name="psop", bufs=2, space="PSUM"
            ) as ps_op, tc.tile_pool(name="expp", bufs=6) as ep, tc.tile_pool(
                name="esump", bufs=2
            ) as esp, tc.tile_pool(name="smallp", bufs=2) as sp, tc.tile_pool(
                name="stagep", bufs=4
            ) as stp:

                def emit_oproj_group(c, jj, nn, eng):
                    op = ps_op.tile([P, SC], F32, name="op", tag="op")
                    i_abs = QH * c + jj
                    for h in range(QH):
                        nc.tensor.matmul(
                            op[:],
                            attnT[(h, c)][:, jj * P : (jj + 1) * P],
                            wo_t[h][:, nn * SC : (nn + 1) * SC],
                            start=(h == 0),
                            stop=(h == QH - 1),
                        )
                    st = stp.tile([P, SC], F16, name="st", tag="st")
                    # alternate eviction between Act and DVE to balance load
                    if eng == 0:
                        nc.scalar.copy(st[:], op[:])
                    else:
                        nc.vector.tensor_copy(st[:], op[:])
                    nc.sync.dma_start(
                        partial[i_abs * P : (i_abs + 1) * P, nn * SC : (nn + 1) * SC],
                        st[:],
                    )

                def oproj_list(c):
                    return [(c, jj, nn) for jj in range(QH) for nn in range(HID // SC)]

                def emit_rs(c):
                    nc.gpsimd.collective_compute(
                        "ReduceScatter",
                        mybir.AluOpType.add,
                        replica_groups=groups,
                        ins=[partial[c * SC : (c + 1) * SC, :]],
                        outs=[rs_out[c * 64 : (c + 1) * 64, :]],
                    )
                    nc.sync.dma_start(
                        out[c * 64 : (c + 1) * 64, :],
                        rs_out[c * 64 : (c + 1) * 64, :],
                    )

                for c in range(NS):
                    prev = oproj_list(c - 1) if c > 0 else []
                    nsk = QH * c + QH  # causal: sk tiles for this chunk
                    total_steps = QH * nsk
                    oi = 0
                    si = 0
                    for h in range(QH):
                        esum = esp.tile([P, SC], F32R, name="esum", tag="esum")
                        pv = ps_pv.tile([P, SC], F32, name="pv", tag="pv")
                        qrhs = qk[(h, c)][:]
                        for t in range(nsk):
                            kc, kj = divmod(t, NS)
                            ktile = qk[(QH, kc)][:, kj * P : (kj + 1) * P]
                            sc_ps = ps_sc.tile([P, SC], F32, name="sc_ps", tag="sc")
                            nc.tensor.matmul(
                                sc_ps[:], ktile, qrhs, start=True, stop=True
                            )
                            e = ep.tile([P, SC], F16, name="e", tag="e")
                            nc.scalar.activation(
                                e[:], sc_ps[:], AF.Exp, scale=ISQRT_D
                            )
                            if t >= QH * c:
                                # mask_t = [zeros(3*P) | upper-tri(P)]; the
                                # right-aligned slice zeroes the fully-masked
                                # prefix and applies the triangular block
                                j = t - QH * c
                                nc.vector.tensor_mul(
                                    e[:, 0 : (j + 1) * P],
                                    e[:, 0 : (j + 1) * P],
                                    mask_t[:, (3 - j) * P : 4 * P],
                                )
                            if t == 0:
                                nc.vector.tensor_copy(esum[:], e[:])
                            else:
                                nc.vector.tensor_add(esum[:], esum[:], e[:])
                            nc.tensor.matmul(
                                pv[:], vnat[t][:], e[:],
                                start=(t == 0), stop=(t == nsk - 1),
                            )
                            si += 1
                            while prev and oi * total_steps < si * len(prev):
                                cc, jj, nn = prev[oi]
                                emit_oproj_group(cc, jj, nn, oi % 2)
                                oi += 1
                        sm = ps_sm.tile([1, SC], F32, name="sm", tag="sm")
                        nc.tensor.matmul(
                            sm[:], ones_t[:], esum[:], start=True, stop=True
                        )
                        rc = sp.tile([1, SC], F32, name="rc", tag="rc")
                        bc = sp.tile([P, SC], F32, name="bc", tag="bc")
                        nc.vector.reciprocal(rc[:], sm[:])
                        nc.gpsimd.partition_broadcast(bc[:], rc[:])
                        nc.vector.tensor_mul(attnT[(h, c)][:], pv[:], bc[:])
                    while oi < len(prev):
                        cc, jj, nn = prev[oi]
                        emit_oproj_group(cc, jj, nn, oi % 2)
                        oi += 1
                    if c > 0:
                        emit_rs(c - 1)

                for idx, (cc, jj, nn) in enumerate(oproj_list(NS - 1)):
                    emit_oproj_group(cc, jj, nn, idx % 2)
                emit_rs(NS - 1)

    nc.compile()
    return nc


def _get_nc():
    if "nc" not in _CACHE:
        _CACHE["nc"] = _build()
    return _CACHE["nc"]


def _host_inputs(positions, hidden_states, Wqkv, Wo):
    """Shard + relayout the full inputs for the 8 cores (fp16 device side)."""
    pos = np.asarray(positions).reshape(-1).astype(np.float64)  # [S]
    hs = np.asarray(hidden_states, dtype=np.float32).reshape(S, HID)
    Wqkv = np.asarray(Wqkv, dtype=np.float32)
    Wo = np.asarray(Wo, dtype=np.float32)

    hT = np.ascontiguousarray(hs.T).astype(np.float16)  # [HID, S]

    half = D // 2
    inv_freq = 1.0 / (THETA ** (np.arange(half, dtype=np.float64) / half))
    ang = pos[None, :] * inv_freq[:, None]  # [64, S]
    cos = np.cos(ang)
    sin = np.sin(ang)
    cos2 = np.ascontiguousarray(np.concatenate([cos, cos], axis=0)).astype(
        np.float16
    )
    sinn2 = np.ascontiguousarray(np.concatenate([-sin, sin], axis=0)).astype(
        np.float16
    )

    # causal mask, [sk, sq] orientation: [zeros(128x384) | upper-tri(128x128)].
    maskd = np.concatenate(
        [np.zeros((P, 3 * P), dtype=np.float16),
         np.triu(np.ones((P, P), dtype=np.float16))], axis=1)
    ident = np.eye(P, dtype=np.float16)
    onesd = np.ones((P, 1), dtype=np.float32)

    qb = Wqkv[:, : H * D]
    kb = Wqkv[:, H * D : H * D + KVH * D]
    vb = Wqkv[:, H * D + KVH * D :]

    in_maps = []
    for c in range(NCORES):
        wq_c = np.concatenate(
            [
                qb[:, c * QH * D : (c + 1) * QH * D],
                kb[:, c * D : (c + 1) * D],
                vb[:, c * D : (c + 1) * D],
            ],
            axis=1,
        ).astype(np.float16)
        wo_c = Wo[c * QH * D : (c + 1) * QH * D, :].astype(np.float16)
        in_maps.append(
            {
                "hT": hT,
                "wqkv": np.ascontiguousarray(wq_c),
                "wo": np.ascontiguousarray(wo_c),
                "cos2": cos2,
                "sinn2": sinn2,
                "maskd": maskd,
                "ident": ident,
                "onesd": onesd,
            }
        )
    return in_maps


def _assemble(results):
    full = np.empty((S, HID), dtype=np.float32)
    for c in range(NCORES):
        oc = np.asarray(results[c]["out"], dtype=np.float32)  # [256, HID]
        for j in range(NS):
            full[SC * j + 64 * c : SC * j + 64 * (c + 1), :] = oc[
                64 * j : 64 * (j + 1), :
            ]
    return full.reshape(1, S, HID)


def kernel(positions, hidden_states, Wqkv, Wo):
    from concourse.bass_utils import run_bass_kernel_spmd

    nc = _get_nc()
    in_maps = _host_inputs(positions, hidden_states, Wqkv, Wo)
    res = run_bass_kernel_spmd(nc, in_maps, core_ids=list(range(NCORES)))
    return _assemble(res.results)


def kernel_timed(positions, hidden_states, Wqkv, Wo, tmpdir="/tmp/ntff_trace"):
    """Like kernel() but with NTFF profiling; returns (output, exec_time_ns)."""
    import os
    import shutil

    from concourse.bass_utils import run_bass_kernel_spmd

    shutil.rmtree(tmpdir, ignore_errors=True)
    os.makedirs(tmpdir, exist_ok=True)
    nc = _get_nc()
    in_maps = _host_inputs(positions, hidden_states, Wqkv, Wo)
    res = run_bass_kernel_spmd(
        nc, in_maps, core_ids=list(range(NCORES)), trace=True, tmpdir=tmpdir
    )
    return _assemble(res.results), res.exec_time_ns


# revision 16
# speedup vs baseline: 1.0545x; 1.0545x over previous
"""Llama attention layer (B=1, S=2048, H=32, KVH=8, D=128, HID=4096) on 8 TRN2
NeuronCores.

Sharding: tensor-parallel over head groups. Core c computes Q heads
[4c..4c+4) and KV head c end-to-end (QKV projection, RoPE, causal GQA
attention, o_proj rows for its heads), then a chunked ReduceScatter sums the
o_proj partials so core c ends up with rows {512j + 64c .. 512j + 64c + 64}
of the output for j in 0..3. The host reassembles the full [2048, 4096]
output by concatenating the shards.

v2 design (fp16 end-to-end, pipelined):
  - All DRAM-resident tensors are fp16 (half the HBM traffic and half the
    collective bytes of fp32; fp16 matmuls run at full PE rate and carry
    10-bit mantissas). PSUM accumulation stays fp32.
  - Single-pass QKV: the whole wqkv shard (6.3 MB fp16) is SBUF-resident,
    hT streams through once. RoPE runs on DVE per chunk, overlapped with
    the next chunk's QKV matmuls.
  - Softmax denominators accumulate on DVE (esum += exp tile) instead of
    per-tile PE ones-matmuls; one [1,512] ones-matmul per (chunk, head)
    reduces esum across partitions.
  - o_proj matmul groups of chunk c-1 are interleaved into the attention
    t-loop of chunk c so the PE never idles waiting on the scalar engine's
    exp tiles.
  - Per-chunk fp16 ReduceScatter overlaps the remaining compute.
"""

import sys

if "/opt/trn_rl_repo" not in sys.path:
    sys.path.insert(0, "/opt/trn_rl_repo")

import numpy as np

# Model dims (hardcoded per problem spec)
H, KVH, D, HID = 32, 8, 128, 4096
S = 2048
THETA = 10000.0
NCORES = 8
QH = H // NCORES          # 4 query heads per core
P = 128                   # partitions
SC = 512                  # sequence chunk (matmul free dim)
NS = S // SC              # 4 chunks
KT = HID // P             # 32 contraction tiles for the projections
ST = S // P               # 16 sequence tiles of 128
NQK = QH + 2              # col-tiles per core in wqkv: q0..q3, k, v
WCOLS = NQK * P           # 768
GK = 4                    # weight k-tiles per DMA group
NG = KT // GK             # 8 groups
ISQRT_D = float(D) ** -0.5

_CACHE = {}


def _build():
    import concourse.bass as bass
    import concourse.tile as tile
    from concourse import bacc, mybir
    from contextlib import ExitStack

    F32 = mybir.dt.float32
    F32R = mybir.dt.float32r
    F16 = mybir.dt.float16
    AF = mybir.ActivationFunctionType

    nc = bacc.Bacc(
        "TRN2",
        target_bir_lowering=False,
        debug=False,
        enable_asserts=True,
        num_devices=NCORES,
    )

    hT = nc.dram_tensor("hT", [HID, S], F16, kind="ExternalInput").ap()
    wqkv = nc.dram_tensor("wqkv", [HID, WCOLS], F16, kind="ExternalInput").ap()
    wo = nc.dram_tensor("wo", [QH * D, HID], F16, kind="ExternalInput").ap()
    cos2 = nc.dram_tensor("cos2", [P, S], F16, kind="ExternalInput").ap()
    sinn2 = nc.dram_tensor("sinn2", [P, S], F16, kind="ExternalInput").ap()
    maskd = nc.dram_tensor("maskd", [P, 4 * P], F16, kind="ExternalInput").ap()
    ident = nc.dram_tensor("ident", [P, P], F16, kind="ExternalInput").ap()
    onesd = nc.dram_tensor("onesd", [P, 1], F16, kind="ExternalInput").ap()
    out = nc.dram_tensor("out", [S // NCORES, HID], F16, kind="ExternalOutput").ap()
    # per-chunk partial / rs tensors: separate DRAM tensors so the
    # whole-tensor WAR tracking never serializes chunk c+1's o_proj DMA
    # writes behind chunk c's in-flight ReduceScatter. The last chunk is
    # split in half to shorten the exposed collective tail.
    partials = [
        nc.dram_tensor(f"partial{c}", [SC, HID], F16).ap() for c in range(NS - 1)
    ]
    partials += [
        nc.dram_tensor("partial3a", [SC // 2, HID], F16).ap(),
        nc.dram_tensor("partial3b", [SC // 2, HID], F16).ap(),
    ]
    rs_outs = [
        nc.dram_tensor(f"rs{c}", [64, HID], F16).ap() for c in range(NS - 1)
    ]
    rs_outs += [
        nc.dram_tensor("rs3a", [32, HID], F16).ap(),
        nc.dram_tensor("rs3b", [32, HID], F16).ap(),
    ]

    groups = [list(range(NCORES))]

    with tile.TileContext(nc) as tc:
        with ExitStack() as ctx:
            # ---------------- constants (whole-kernel lifetime) ----------------
            cpool = ctx.enter_context(tc.tile_pool(name="const", bufs=1))
            mask_t = cpool.tile([P, 4 * P], F16, name="mask_t")
            ident_t = cpool.tile([P, P], F16, name="ident_t")
            ones_t = cpool.tile([P, 1], F16, name="ones_t")
            cos_t = cpool.tile([P, S], F16, name="cos_t")
            sin_t = cpool.tile([P, S], F16, name="sin_t")

            # ------------- persistent activation buffers -------------
            ppool = ctx.enter_context(tc.tile_pool(name="persist", bufs=1))
            qk = {}
            for n in range(QH + 1):
                for c in range(NS):
                    qk[(n, c)] = ppool.tile(
                        [P, SC], F16, name=f"qk{n}_{c}", tag=f"qk{n}_{c}"
                    )
            vT = [
                ppool.tile([P, SC], F16, name=f"vT{c}", tag=f"vT{c}")
                for c in range(NS)
            ]
            vnat = [
                ppool.tile([P, P], F16, name=f"vn{t}", tag=f"vn{t}")
                for t in range(ST)
            ]
            attnT = {}
            for h in range(QH):
                for c in range(NS):
                    attnT[(h, c)] = ppool.tile(
                        [P, SC], F16, name=f"at{h}_{c}", tag=f"at{h}_{c}"
                    )

            # ------------- resident weights (wqkv + wo, fp16) -------------
            # wqkv groups load first (phase A needs them immediately); the
            # small constants follow; wo loads are deferred until after
            # phase A emission so they don't delay the QKV pipeline.
            wpool = ctx.enter_context(tc.tile_pool(name="wres", bufs=1))
            wo_t = [
                wpool.tile([P, HID], F16, name=f"wo{hh}", tag=f"wo{hh}")
                for hh in range(QH)
            ]
            wq_g = {}

            def load_group(g):
                wq_g[g] = wpool.tile(
                    [P, GK * WCOLS], F16, name=f"wqg{g}", tag=f"wqg{g}"
                )
                src = wqkv[g * GK * P : (g + 1) * GK * P, :].rearrange(
                    "(t p) n -> p t n", p=P
                )
                dst = wq_g[g][:].rearrange("p (t n) -> p t n", t=GK)
                nc.sync.dma_start(dst, src)

            def wslice(k, n):
                g, kk = divmod(k, GK)
                off = kk * WCOLS + n * P
                return wq_g[g][:, off : off + P]

            for g in range(NG):
                load_group(g)
            nc.sync.dma_start(mask_t[:], maskd)
            nc.sync.dma_start(ident_t[:], ident)
            nc.sync.dma_start(ones_t[:], onesd)
            nc.sync.dma_start(cos_t[:], cos2)
            nc.sync.dma_start(sin_t[:], sinn2)

            # ---------------- stage A: QKV projection + RoPE + vT ------------
            half = P // 2
            with tc.tile_pool(name="ht", bufs=4) as h_pool, tc.tile_pool(
                name="psA", bufs=NQK, space="PSUM"
            ) as psA, tc.tile_pool(
                name="pst", bufs=2, space="PSUM"
            ) as pst, tc.tile_pool(name="ropet", bufs=3) as rpool:
                for c in range(NS):
                    ps = [
                        psA.tile([P, SC], F32, name=f"psA{n}", tag="psA")
                        for n in range(NQK)
                    ]
                    for k in range(KT):
                        ht_t = h_pool.tile([P, SC], F16, name="ht_t", tag="ht")
                        nc.sync.dma_start(
                            ht_t[:], hT[k * P : (k + 1) * P, c * SC : (c + 1) * SC]
                        )
                        for n in range(NQK):
                            nc.tensor.matmul(
                                ps[n][:],
                                wslice(k, n),
                                ht_t[:],
                                start=(k == 0),
                                stop=(k == KT - 1),
                            )
                    for n in range(QH + 1):
                        nc.scalar.copy(qk[(n, c)][:], ps[n][:])
                    nc.scalar.copy(vT[c][:], ps[NQK - 1][:])

                    # RoPE in place on this chunk's q heads and k (DVE),
                    # overlapped with the next chunk's QKV matmuls
                    csl = cos_t[:, c * SC : (c + 1) * SC]
                    ssl = sin_t[:, c * SC : (c + 1) * SC]
                    for n in range(QH + 1):
                        src = qk[(n, c)]
                        swp = rpool.tile([P, SC], F16, name="swp", tag="swp")
                        t1 = rpool.tile([P, SC], F16, name="t1", tag="t1")
                        nc.sync.dma_start(swp[0:half, :], src[half:P, :])
                        nc.sync.dma_start(swp[half:P, :], src[0:half, :])
                        nc.vector.tensor_mul(t1[:], src[:], csl)
                        nc.vector.tensor_mul(swp[:], swp[:], ssl)
                        nc.vector.tensor_add(src[:], t1[:], swp[:])

                # v transpose to natural [s, d]
                for t in range(ST):
                    c, j = divmod(t, NS)
                    tp = pst.tile([P, P], F16, name="tp", tag="tp")
                    nc.tensor.transpose(
                        tp[:], vT[c][:, j * P : (j + 1) * P], ident_t[:]
                    )
                    nc.scalar.copy(vnat[t][:], tp[:])

            # wo loads queue behind phase A's DMA stream (needed ~200us in)
            for hh in range(QH):
                nc.sync.dma_start(wo_t[hh][:], wo[hh * P : (hh + 1) * P, :])

            # ---------- attention + o_proj + reduce-scatter ----------
            with tc.tile_pool(name="pssc", bufs=3, space="PSUM") as ps_sc, tc.tile_pool(
                name="pssm", bufs=1, space="PSUM"
            ) as ps_sm, tc.tile_pool(
                name="pspv", bufs=2, space="PSUM"
            ) as ps_pv, tc.tile_pool(
                name="psop", bufs=2, space="PSUM"
            ) as ps_op, tc.tile_pool(name="expp", bufs=6) as ep, tc.tile_pool(
                name="esump", bufs=2
            ) as esp, tc.tile_pool(name="smallp", bufs=2) as sp, tc.tile_pool(
                name="stagep", bufs=4
            ) as stp:

                def emit_oproj_group(c, jj, nn, eng):
                    op = ps_op.tile([P, SC], F32, name="op", tag="op")
                    for h in range(QH):
                        nc.tensor.matmul(
                            op[:],
                            attnT[(h, c)][:, jj * P : (jj + 1) * P],
                            wo_t[h][:, nn * SC : (nn + 1) * SC],
                            start=(h == 0),
                            stop=(h == QH - 1),
                        )
                    st = stp.tile([P, SC], F16, name="st", tag="st")
                    # alternate eviction between Act and DVE to balance load
                    if eng == 0:
                        nc.scalar.copy(st[:], op[:])
                    else:
                        nc.vector.tensor_copy(st[:], op[:])
                    if c < NS - 1:
                        dst = partials[c]
                        row = jj * P
                    else:
                        dst = partials[NS - 1 + jj // 2]
                        row = (jj % 2) * P
                    nc.sync.dma_start(
                        dst[row : row + P, nn * SC : (nn + 1) * SC], st[:]
                    )

                def oproj_list(c):
                    return [(c, jj, nn) for jj in range(QH) for nn in range(HID // SC)]

                def emit_rs(idx):
                    # idx 0..2: full chunks; idx 3,4: halves of chunk 3
                    rows = 64 if idx < NS - 1 else 32
                    off = idx * 64 if idx < NS - 1 else 192 + (idx - NS + 1) * 32
                    nc.gpsimd.collective_compute(
                        "ReduceScatter",
                        mybir.AluOpType.add,
                        replica_groups=groups,
                        ins=[partials[idx][:, :]],
                        outs=[rs_outs[idx][:, :]],
                    )
                    nc.sync.dma_start(out[off : off + rows, :], rs_outs[idx][:, :])

                for c in range(NS):
                    prev = oproj_list(c - 1) if c > 0 else []
                    nsk = QH * c + QH  # causal: sk tiles for this chunk
                    total_steps = QH * nsk
                    oi = 0
                    si = 0
                    for h in range(QH):
                        esum = esp.tile([P, SC], F16, name="esum", tag="esum")
                        pv = ps_pv.tile([P, SC], F32, name="pv", tag="pv")
                        qrhs = qk[(h, c)][:]
                        for t in range(nsk):
                            kc, kj = divmod(t, NS)
                            ktile = qk[(QH, kc)][:, kj * P : (kj + 1) * P]
                            # diagonal tiles: columns below j*P are fully
                            # masked; compute only the live region
                            lo = (t - QH * c) * P if t >= QH * c else 0
                            sc_ps = ps_sc.tile([P, SC], F32, name="sc_ps", tag="sc")
                            nc.tensor.matmul(
                                sc_ps[:, lo:SC], ktile, qrhs[:, lo:SC],
                                start=True, stop=True,
                            )
                            e = ep.tile([P, SC], F16, name="e", tag="e")
                            nc.scalar.activation(
                                e[:, lo:SC], sc_ps[:, lo:SC], AF.Exp, scale=ISQRT_D
                            )
                            if t >= QH * c:
                                # triangular mask on the diagonal P-block
                                nc.vector.tensor_mul(
                                    e[:, lo : lo + P],
                                    e[:, lo : lo + P],
                                    mask_t[:, 3 * P : 4 * P],
                                )
                            if t == 0:
                                nc.vector.tensor_copy(esum[:], e[:])
                            else:
                                nc.vector.tensor_add(
                                    esum[:, lo:SC], esum[:, lo:SC], e[:, lo:SC]
                                )
                            nc.tensor.matmul(
                                pv[:, lo:SC], vnat[t][:], e[:, lo:SC],
                                start=(t == 0), stop=(t == nsk - 1),
                            )
                            si += 1
                            while prev and oi * total_steps < si * len(prev):
                                cc, jj, nn = prev[oi]
                                emit_oproj_group(cc, jj, nn, oi % 2)
                                oi += 1
                        sm = ps_sm.tile([1, SC], F32, name="sm", tag="sm")
                        nc.tensor.matmul(
                            sm[:], ones_t[:], esum[:], start=True, stop=True
                        )
                        rc = sp.tile([1, SC], F32, name="rc", tag="rc")
                        bc = sp.tile([P, SC], F32, name="bc", tag="bc")
                        nc.vector.reciprocal(rc[:], sm[:])
                        nc.gpsimd.partition_broadcast(bc[:], rc[:])
                        nc.vector.tensor_mul(attnT[(h, c)][:], pv[:], bc[:])
                    while oi < len(prev):
                        cc, jj, nn = prev[oi]
                        emit_oproj_group(cc, jj, nn, oi % 2)
                        oi += 1
                    if c > 0:
                        emit_rs(c - 1)

                last = oproj_list(NS - 1)
                for idx, (cc, jj, nn) in enumerate(last):
                    emit_oproj_group(cc, jj, nn, idx % 2)
                    if idx == len(last) // 2 - 1:
                        emit_rs(NS - 1)  # first half of chunk 3
                emit_rs(NS)  # second half of chunk 3

    nc.compile()
    return nc


def _get_nc():
    if "nc" not in _CACHE:
        _CACHE["nc"] = _build()
    return _CACHE["nc"]


def _host_inputs(positions, hidden_states, Wqkv, Wo):
    """Shard + relayout the full inputs for the 8 cores (fp16 device side)."""
    pos = np.asarray(positions).reshape(-1).astype(np.float64)  # [S]
    hs = np.asarray(hidden_states, dtype=np.float32).reshape(S, HID)
    Wqkv = np.asarray(Wqkv, dtype=np.float32)
    Wo = np.asarray(Wo, dtype=np.float32)

    hT = np.ascontiguousarray(hs.T).astype(np.float16)  # [HID, S]

    half = D // 2
    inv_freq = 1.0 / (THETA ** (np.arange(half, dtype=np.float64) / half))
    ang = pos[None, :] * inv_freq[:, None]  # [64, S]
    cos = np.cos(ang)
    sin = np.sin(ang)
    cos2 = np.ascontiguousarray(np.concatenate([cos, cos], axis=0)).astype(
        np.float16
    )
    sinn2 = np.ascontiguousarray(np.concatenate([-sin, sin], axis=0)).astype(
        np.float16
    )

    # causal mask, [sk, sq] orientation: [zeros(128x384) | upper-tri(128x128)].
    maskd = np.concatenate(
        [np.zeros((P, 3 * P), dtype=np.float16),
         np.triu(np.ones((P, P), dtype=np.float16))], axis=1)
    ident = np.eye(P, dtype=np.float16)
    onesd = np.ones((P, 1), dtype=np.float16)

    qb = Wqkv[:, : H * D]
    kb = Wqkv[:, H * D : H * D + KVH * D]
    vb = Wqkv[:, H * D + KVH * D :]

    in_maps = []
    for c in range(NCORES):
        wq_c = np.concatenate(
            [
                qb[:, c * QH * D : (c + 1) * QH * D],
                kb[:, c * D : (c + 1) * D],
                vb[:, c * D : (c + 1) * D],
            ],
            axis=1,
        ).astype(np.float16)
        wo_c = Wo[c * QH * D : (c + 1) * QH * D, :].astype(np.float16)
        in_maps.append(
            {
                "hT": hT,
                "wqkv": np.ascontiguousarray(wq_c),
                "wo": np.ascontiguousarray(wo_c),
                "cos2": cos2,
                "sinn2": sinn2,
                "maskd": maskd,
                "ident": ident,
                "onesd": onesd,
            }
        )
    return in_maps


def _assemble(results):
    full = np.empty((S, HID), dtype=np.float32)
    for c in range(NCORES):
        oc = np.asarray(results[c]["out"], dtype=np.float32)  # [256, HID]
        for j in range(NS - 1):
            full[SC * j + 64 * c : SC * j + 64 * (c + 1), :] = oc[
                64 * j : 64 * (j + 1), :
            ]
        # chunk 3 was reduce-scattered as two 256-row halves
        full[3 * SC + 32 * c : 3 * SC + 32 * (c + 1), :] = oc[192:224, :]
        full[3 * SC + 256 + 32 * c : 3 * SC + 256 + 32 * (c + 1), :] = oc[
            224:256, :
        ]
    return full.reshape(1, S, HID)


def kernel(positions, hidden_states, Wqkv, Wo):
    from concourse.bass_utils import run_bass_kernel_spmd

    nc = _get_nc()
    in_maps = _host_inputs(positions, hidden_states, Wqkv, Wo)
    res = run_bass_kernel_spmd(nc, in_maps, core_ids=list(range(NCORES)))
    return _assemble(res.results)


def kernel_timed(positions, hidden_states, Wqkv, Wo, tmpdir="/tmp/ntff_trace"):
    """Like kernel() but with NTFF profiling; returns (output, exec_time_ns)."""
    import os
    import shutil

    from concourse.bass_utils import run_bass_kernel_spmd

    shutil.rmtree(tmpdir, ignore_errors=True)
    os.makedirs(tmpdir, exist_ok=True)
    nc = _get_nc()
    in_maps = _host_inputs(positions, hidden_states, Wqkv, Wo)
    res = run_bass_kernel_spmd(
        nc, in_maps, core_ids=list(range(NCORES)), trace=True, tmpdir=tmpdir
    )
    return _assemble(res.results), res.exec_time_ns


# revision 19
# speedup vs baseline: 1.0649x; 1.0098x over previous
"""Llama attention layer (B=1, S=2048, H=32, KVH=8, D=128, HID=4096) on 8 TRN2
NeuronCores.

Sharding: tensor-parallel over head groups. Core c computes Q heads
[4c..4c+4) and KV head c end-to-end (QKV projection, RoPE, causal GQA
attention, o_proj rows for its heads), then a chunked ReduceScatter sums the
o_proj partials so core c ends up with rows {512j + 64c .. 512j + 64c + 64}
of the output for j in 0..3. The host reassembles the full [2048, 4096]
output by concatenating the shards.

v2 design (fp16 end-to-end, pipelined):
  - All DRAM-resident tensors are fp16 (half the HBM traffic and half the
    collective bytes of fp32; fp16 matmuls run at full PE rate and carry
    10-bit mantissas). PSUM accumulation stays fp32.
  - Single-pass QKV: the whole wqkv shard (6.3 MB fp16) is SBUF-resident,
    hT streams through once. RoPE runs on DVE per chunk, overlapped with
    the next chunk's QKV matmuls.
  - Softmax denominators accumulate on DVE (esum += exp tile) instead of
    per-tile PE ones-matmuls; one [1,512] ones-matmul per (chunk, head)
    reduces esum across partitions.
  - o_proj matmul groups of chunk c-1 are interleaved into the attention
    t-loop of chunk c so the PE never idles waiting on the scalar engine's
    exp tiles.
  - Per-chunk fp16 ReduceScatter overlaps the remaining compute.
"""

import sys

if "/opt/trn_rl_repo" not in sys.path:
    sys.path.insert(0, "/opt/trn_rl_repo")

import numpy as np

# Model dims (hardcoded per problem spec)
H, KVH, D, HID = 32, 8, 128, 4096
S = 2048
THETA = 10000.0
NCORES = 8
QH = H // NCORES          # 4 query heads per core
P = 128                   # partitions
SC = 512                  # sequence chunk (matmul free dim)
NS = S // SC              # 4 chunks
KT = HID // P             # 32 contraction tiles for the projections
ST = S // P               # 16 sequence tiles of 128
NQK = QH + 2              # col-tiles per core in wqkv: q0..q3, k, v
WCOLS = NQK * P           # 768
GK = 4                    # weight k-tiles per DMA group
NG = KT // GK             # 8 groups
ISQRT_D = float(D) ** -0.5

_CACHE = {}


def _build():
    import concourse.bass as bass
    import concourse.tile as tile
    from concourse import bacc, mybir
    from contextlib import ExitStack

    F32 = mybir.dt.float32
    F32R = mybir.dt.float32r
    F16 = mybir.dt.float16
    AF = mybir.ActivationFunctionType

    nc = bacc.Bacc(
        "TRN2",
        target_bir_lowering=False,
        debug=False,
        enable_asserts=True,
        num_devices=NCORES,
    )

    hT = nc.dram_tensor("hT", [HID, S], F16, kind="ExternalInput").ap()
    wqkv = nc.dram_tensor("wqkv", [HID, WCOLS], F16, kind="ExternalInput").ap()
    wo = nc.dram_tensor("wo", [QH * D, HID], F16, kind="ExternalInput").ap()
    cos2 = nc.dram_tensor("cos2", [P, S], F16, kind="ExternalInput").ap()
    sinn2 = nc.dram_tensor("sinn2", [P, S], F16, kind="ExternalInput").ap()
    maskd = nc.dram_tensor("maskd", [P, 4 * P], F16, kind="ExternalInput").ap()
    ident = nc.dram_tensor("ident", [P, P], F16, kind="ExternalInput").ap()
    onesd = nc.dram_tensor("onesd", [P, 1], F16, kind="ExternalInput").ap()
    out = nc.dram_tensor("out", [S // NCORES, HID], F16, kind="ExternalOutput").ap()
    # per-chunk partial / rs tensors: separate DRAM tensors so the
    # whole-tensor WAR tracking never serializes chunk c+1's o_proj DMA
    # writes behind chunk c's in-flight ReduceScatter. The last chunk is
    # split in half to shorten the exposed collective tail.
    partials = [
        nc.dram_tensor(f"partial{c}", [SC, HID], F16).ap() for c in range(NS - 1)
    ]
    partials += [
        nc.dram_tensor("partial3a", [SC // 2, HID], F16).ap(),
        nc.dram_tensor("partial3b", [SC // 2, HID], F16).ap(),
    ]
    rs_outs = [
        nc.dram_tensor(f"rs{c}", [64, HID], F16).ap() for c in range(NS - 1)
    ]
    rs_outs += [
        nc.dram_tensor("rs3a", [32, HID], F16).ap(),
        nc.dram_tensor("rs3b", [32, HID], F16).ap(),
    ]

    groups = [list(range(NCORES))]

    with tile.TileContext(nc) as tc:
        with ExitStack() as ctx:
            # ---------------- constants (whole-kernel lifetime) ----------------
            cpool = ctx.enter_context(tc.tile_pool(name="const", bufs=1))
            mask_t = cpool.tile([P, 4 * P], F16, name="mask_t")
            ident_t = cpool.tile([P, P], F16, name="ident_t")
            ones_t = cpool.tile([P, 1], F16, name="ones_t")
            cos_t = cpool.tile([P, S], F16, name="cos_t")
            sin_t = cpool.tile([P, S], F16, name="sin_t")

            # ------------- persistent activation buffers -------------
            ppool = ctx.enter_context(tc.tile_pool(name="persist", bufs=1))
            qk = {}
            for n in range(QH + 1):
                for c in range(NS):
                    qk[(n, c)] = ppool.tile(
                        [P, SC], F16, name=f"qk{n}_{c}", tag=f"qk{n}_{c}"
                    )
            vT = [
                ppool.tile([P, SC], F16, name=f"vT{c}", tag=f"vT{c}")
                for c in range(NS)
            ]
            vnat = [
                ppool.tile([P, P], F16, name=f"vn{t}", tag=f"vn{t}")
                for t in range(ST)
            ]
            attnT = {}
            for h in range(QH):
                for c in range(NS):
                    attnT[(h, c)] = ppool.tile(
                        [P, SC], F16, name=f"at{h}_{c}", tag=f"at{h}_{c}"
                    )

            # ------------- resident weights (wqkv + wo, fp16) -------------
            # wqkv groups load first (phase A needs them immediately); the
            # small constants follow; wo loads are deferred until after
            # phase A emission so they don't delay the QKV pipeline.
            wpool = ctx.enter_context(tc.tile_pool(name="wres", bufs=1))
            wo_t = [
                wpool.tile([P, HID], F16, name=f"wo{hh}", tag=f"wo{hh}")
                for hh in range(QH)
            ]
            wq_g = {}

            def load_group(g):
                wq_g[g] = wpool.tile(
                    [P, GK * WCOLS], F16, name=f"wqg{g}", tag=f"wqg{g}"
                )
                src = wqkv[g * GK * P : (g + 1) * GK * P, :].rearrange(
                    "(t p) n -> p t n", p=P
                )
                dst = wq_g[g][:].rearrange("p (t n) -> p t n", t=GK)
                nc.sync.dma_start(dst, src)

            def wslice(k, n):
                g, kk = divmod(k, GK)
                off = kk * WCOLS + n * P
                return wq_g[g][:, off : off + P]

            for g in range(NG):
                load_group(g)
            nc.sync.dma_start(mask_t[:], maskd)
            nc.sync.dma_start(ident_t[:], ident)
            nc.sync.dma_start(ones_t[:], onesd)
            nc.sync.dma_start(cos_t[:], cos2)
            nc.sync.dma_start(sin_t[:], sinn2)

            # ---------------- stage A: QKV projection + RoPE + vT ------------
            half = P // 2
            with tc.tile_pool(name="ht", bufs=4) as h_pool, tc.tile_pool(
                name="psA", bufs=NQK, space="PSUM"
            ) as psA, tc.tile_pool(
                name="pst", bufs=2, space="PSUM"
            ) as pst, tc.tile_pool(name="ropet", bufs=3) as rpool:
                for c in range(NS):
                    ps = [
                        psA.tile([P, SC], F32, name=f"psA{n}", tag="psA")
                        for n in range(NQK)
                    ]
                    for k in range(KT):
                        ht_t = h_pool.tile([P, SC], F16, name="ht_t", tag="ht")
                        nc.sync.dma_start(
                            ht_t[:], hT[k * P : (k + 1) * P, c * SC : (c + 1) * SC]
                        )
                        for n in range(NQK):
                            nc.tensor.matmul(
                                ps[n][:],
                                wslice(k, n),
                                ht_t[:],
                                start=(k == 0),
                                stop=(k == KT - 1),
                            )
                    for n in range(QH + 1):
                        nc.scalar.copy(qk[(n, c)][:], ps[n][:])
                    nc.scalar.copy(vT[c][:], ps[NQK - 1][:])

                    # RoPE in place on this chunk's q heads and k (DVE),
                    # overlapped with the next chunk's QKV matmuls
                    csl = cos_t[:, c * SC : (c + 1) * SC]
                    ssl = sin_t[:, c * SC : (c + 1) * SC]
                    for n in range(QH + 1):
                        src = qk[(n, c)]
                        swp = rpool.tile([P, SC], F16, name="swp", tag="swp")
                        t1 = rpool.tile([P, SC], F16, name="t1", tag="t1")
                        nc.sync.dma_start(swp[0:half, :], src[half:P, :])
                        nc.sync.dma_start(swp[half:P, :], src[0:half, :])
                        nc.vector.tensor_mul(t1[:], src[:], csl)
                        nc.vector.tensor_mul(swp[:], swp[:], ssl)
                        nc.vector.tensor_add(src[:], t1[:], swp[:])

                # v transpose to natural [s, d]
                for t in range(ST):
                    c, j = divmod(t, NS)
                    tp = pst.tile([P, P], F16, name="tp", tag="tp")
                    nc.tensor.transpose(
                        tp[:], vT[c][:, j * P : (j + 1) * P], ident_t[:]
                    )
                    nc.scalar.copy(vnat[t][:], tp[:])

            # wo loads queue behind phase A's DMA stream (needed ~200us in)
            for hh in range(QH):
                nc.sync.dma_start(wo_t[hh][:], wo[hh * P : (hh + 1) * P, :])

            # ---------- attention + o_proj + reduce-scatter ----------
            with tc.tile_pool(name="pssc", bufs=3, space="PSUM") as ps_sc, tc.tile_pool(
                name="pssm", bufs=1, space="PSUM"
            ) as ps_sm, tc.tile_pool(
                name="pspv", bufs=2, space="PSUM"
            ) as ps_pv, tc.tile_pool(
                name="psop", bufs=2, space="PSUM"
            ) as ps_op, tc.tile_pool(name="expp", bufs=6) as ep, tc.tile_pool(
                name="esump", bufs=2
            ) as esp, tc.tile_pool(name="smallp", bufs=2) as sp, tc.tile_pool(
                name="stagep", bufs=32
            ) as stp:
                # stagep is deep on purpose: while a ReduceScatter is in
                # flight the SDMA engines starve regular DMA queues, so a
                # full chunk of partial-write DMAs (32 tiles) must be able
                # to back up without blocking the eviction engines.

                def emit_oproj_group(c, jj, nn, eng):
                    op = ps_op.tile([P, SC], F32, name="op", tag="op")
                    for h in range(QH):
                        nc.tensor.matmul(
                            op[:],
                            attnT[(h, c)][:, jj * P : (jj + 1) * P],
                            wo_t[h][:, nn * SC : (nn + 1) * SC],
                            start=(h == 0),
                            stop=(h == QH - 1),
                        )
                    st = stp.tile([P, SC], F16, name="st", tag="st")
                    # alternate eviction between Act and DVE to balance load
                    if eng == 0:
                        nc.scalar.copy(st[:], op[:])
                    else:
                        nc.vector.tensor_copy(st[:], op[:])
                    if c < NS - 1:
                        dst = partials[c]
                        row = jj * P
                    else:
                        dst = partials[NS - 1 + jj // 2]
                        row = (jj % 2) * P
                    nc.sync.dma_start(
                        dst[row : row + P, nn * SC : (nn + 1) * SC], st[:]
                    )

                def oproj_list(c):
                    return [(c, jj, nn) for jj in range(QH) for nn in range(HID // SC)]

                def emit_rs(idx):
                    # idx 0..2: full chunks; idx 3,4: halves of chunk 3
                    rows = 64 if idx < NS - 1 else 32
                    off = idx * 64 if idx < NS - 1 else 192 + (idx - NS + 1) * 32
                    nc.gpsimd.collective_compute(
                        "ReduceScatter",
                        mybir.AluOpType.add,
                        replica_groups=groups,
                        ins=[partials[idx][:, :]],
                        outs=[rs_outs[idx][:, :]],
                    )
                    nc.sync.dma_start(out[off : off + rows, :], rs_outs[idx][:, :])

                for c in range(NS):
                    prev = oproj_list(c - 1) if c > 0 else []
                    nsk = QH * c + QH  # causal: sk tiles for this chunk
                    total_steps = QH * nsk
                    oi = 0
                    si = 0
                    for h in range(QH):
                        esum = esp.tile([P, SC], F16, name="esum", tag="esum")
                        pv = ps_pv.tile([P, SC], F32, name="pv", tag="pv")
                        qrhs = qk[(h, c)][:]
                        for t in range(nsk):
                            kc, kj = divmod(t, NS)
                            ktile = qk[(QH, kc)][:, kj * P : (kj + 1) * P]
                            # diagonal tiles: columns below j*P are fully
                            # masked; compute only the live region
                            lo = (t - QH * c) * P if t >= QH * c else 0
                            sc_ps = ps_sc.tile([P, SC], F32, name="sc_ps", tag="sc")
                            nc.tensor.matmul(
                                sc_ps[:, lo:SC], ktile, qrhs[:, lo:SC],
                                start=True, stop=True,
                            )
                            e = ep.tile([P, SC], F16, name="e", tag="e")
                            nc.scalar.activation(
                                e[:, lo:SC], sc_ps[:, lo:SC], AF.Exp, scale=ISQRT_D
                            )
                            if t >= QH * c:
                                # triangular mask on the diagonal P-block
                                nc.vector.tensor_mul(
                                    e[:, lo : lo + P],
                                    e[:, lo : lo + P],
                                    mask_t[:, 3 * P : 4 * P],
                                )
                            if t == 0:
                                nc.vector.tensor_copy(esum[:], e[:])
                            else:
                                nc.vector.tensor_add(
                                    esum[:, lo:SC], esum[:, lo:SC], e[:, lo:SC]
                                )
                            nc.tensor.matmul(
                                pv[:, lo:SC], vnat[t][:], e[:, lo:SC],
                                start=(t == 0), stop=(t == nsk - 1),
                            )
                            si += 1
                            while prev and oi * total_steps < si * len(prev):
                                cc, jj, nn = prev[oi]
                                emit_oproj_group(cc, jj, nn, oi % 2)
                                oi += 1
                        sm = ps_sm.tile([1, SC], F32, name="sm", tag="sm")
                        nc.tensor.matmul(
                            sm[:], ones_t[:], esum[:], start=True, stop=True
                        )
                        # broadcast first, then reciprocal on [128,512] so
                        # the DVE uses all lanes (a [1,512] reciprocal is
                        # 1-lane and takes 3.3us)
                        smh = sp.tile([1, SC], F16, name="smh", tag="smh")
                        bc = sp.tile([P, SC], F16, name="bc", tag="bc")
                        nc.scalar.copy(smh[:], sm[:])
                        nc.gpsimd.partition_broadcast(bc[:], smh[:])
                        with nc.allow_low_precision(
                            reason="softmax denom recip in fp16; denom in "
                            "[1, 5.5e3] so fp16 rel err ~5e-4 vs 2e-2 budget"
                        ):
                            nc.vector.reciprocal(bc[:], bc[:])
                        nc.vector.tensor_mul(attnT[(h, c)][:], pv[:], bc[:])
                    while oi < len(prev):
                        cc, jj, nn = prev[oi]
                        emit_oproj_group(cc, jj, nn, oi % 2)
                        oi += 1
                    if c > 0:
                        emit_rs(c - 1)

                last = oproj_list(NS - 1)
                for idx, (cc, jj, nn) in enumerate(last):
                    emit_oproj_group(cc, jj, nn, idx % 2)
                    if idx == len(last) // 2 - 1:
                        emit_rs(NS - 1)  # first half of chunk 3
                emit_rs(NS)  # second half of chunk 3

    nc.compile()
    return nc


def _get_nc():
    if "nc" not in _CACHE:
        _CACHE["nc"] = _build()
    return _CACHE["nc"]


def _host_inputs(positions, hidden_states, Wqkv, Wo):
    """Shard + relayout the full inputs for the 8 cores (fp16 device side)."""
    pos = np.asarray(positions).reshape(-1).astype(np.float64)  # [S]
    hs = np.asarray(hidden_states, dtype=np.float32).reshape(S, HID)
    Wqkv = np.asarray(Wqkv, dtype=np.float32)
    Wo = np.asarray(Wo, dtype=np.float32)

    hT = np.ascontiguousarray(hs.T).astype(np.float16)  # [HID, S]

    half = D // 2
    inv_freq = 1.0 / (THETA ** (np.arange(half, dtype=np.float64) / half))
    ang = pos[None, :] * inv_freq[:, None]  # [64, S]
    cos = np.cos(ang)
    sin = np.sin(ang)
    cos2 = np.ascontiguousarray(np.concatenate([cos, cos], axis=0)).astype(
        np.float16
    )
    sinn2 = np.ascontiguousarray(np.concatenate([-sin, sin], axis=0)).astype(
        np.float16
    )

    # causal mask, [sk, sq] orientation: [zeros(128x384) | upper-tri(128x128)].
    maskd = np.concatenate(
        [np.zeros((P, 3 * P), dtype=np.float16),
         np.triu(np.ones((P, P), dtype=np.float16))], axis=1)
    ident = np.eye(P, dtype=np.float16)
    onesd = np.ones((P, 1), dtype=np.float16)

    qb = Wqkv[:, : H * D]
    kb = Wqkv[:, H * D : H * D + KVH * D]
    vb = Wqkv[:, H * D + KVH * D :]

    in_maps = []
    for c in range(NCORES):
        wq_c = np.concatenate(
            [
                qb[:, c * QH * D : (c + 1) * QH * D],
                kb[:, c * D : (c + 1) * D],
                vb[:, c * D : (c + 1) * D],
            ],
            axis=1,
        ).astype(np.float16)
        wo_c = Wo[c * QH * D : (c + 1) * QH * D, :].astype(np.float16)
        in_maps.append(
            {
                "hT": hT,
                "wqkv": np.ascontiguousarray(wq_c),
                "wo": np.ascontiguousarray(wo_c),
                "cos2": cos2,
                "sinn2": sinn2,
                "maskd": maskd,
                "ident": ident,
                "onesd": onesd,
            }
        )
    return in_maps


def _assemble(results):
    full = np.empty((S, HID), dtype=np.float32)
    for c in range(NCORES):
        oc = np.asarray(results[c]["out"], dtype=np.float32)  # [256, HID]
        for j in range(NS - 1):
            full[SC * j + 64 * c : SC * j + 64 * (c + 1), :] = oc[
                64 * j : 64 * (j + 1), :
            ]
        # chunk 3 was reduce-scattered as two 256-row halves
        full[3 * SC + 32 * c : 3 * SC + 32 * (c + 1), :] = oc[192:224, :]
        full[3 * SC + 256 + 32 * c : 3 * SC + 256 + 32 * (c + 1), :] = oc[
            224:256, :
        ]
    return full.reshape(1, S, HID)


def kernel(positions, hidden_states, Wqkv, Wo):
    from concourse.bass_utils import run_bass_kernel_spmd

    nc = _get_nc()
    in_maps = _host_inputs(positions, hidden_states, Wqkv, Wo)
    res = run_bass_kernel_spmd(nc, in_maps, core_ids=list(range(NCORES)))
    return _assemble(res.results)


def kernel_timed(positions, hidden_states, Wqkv, Wo, tmpdir="/tmp/ntff_trace"):
    """Like kernel() but with NTFF profiling; returns (output, exec_time_ns)."""
    import os
    import shutil

    from concourse.bass_utils import run_bass_kernel_spmd

    shutil.rmtree(tmpdir, ignore_errors=True)
    os.makedirs(tmpdir, exist_ok=True)
    nc = _get_nc()
    in_maps = _host_inputs(positions, hidden_states, Wqkv, Wo)
    res = run_bass_kernel_spmd(
        nc, in_maps, core_ids=list(range(NCORES)), trace=True, tmpdir=tmpdir
    )
    return _assemble(res.results), res.exec_time_ns


# revision 23
# speedup vs baseline: 1.1722x; 1.1008x over previous
"""Llama attention layer (B=1, S=2048, H=32, KVH=8, D=128, HID=4096) on 8 TRN2
NeuronCores.

Sharding: tensor-parallel over head groups. Core c computes Q heads
[4c..4c+4) and KV head c end-to-end (QKV projection, RoPE, causal GQA
attention, o_proj rows for its heads), then a chunked ReduceScatter sums the
o_proj partials so core c ends up with rows {512j + 64c .. 512j + 64c + 64}
of the output for j in 0..3. The host reassembles the full [2048, 4096]
output by concatenating the shards.

v2 design (fp16 end-to-end, pipelined):
  - All DRAM-resident tensors are fp16 (half the HBM traffic and half the
    collective bytes of fp32; fp16 matmuls run at full PE rate and carry
    10-bit mantissas). PSUM accumulation stays fp32.
  - Single-pass QKV: the whole wqkv shard (6.3 MB fp16) is SBUF-resident,
    hT streams through once. RoPE runs on DVE per chunk, overlapped with
    the next chunk's QKV matmuls.
  - Softmax denominators accumulate on DVE (esum += exp tile) instead of
    per-tile PE ones-matmuls; one [1,512] ones-matmul per (chunk, head)
    reduces esum across partitions.
  - o_proj matmul groups of chunk c-1 are interleaved into the attention
    t-loop of chunk c so the PE never idles waiting on the scalar engine's
    exp tiles.
  - Per-chunk fp16 ReduceScatter overlaps the remaining compute.
"""

import sys

if "/opt/trn_rl_repo" not in sys.path:
    sys.path.insert(0, "/opt/trn_rl_repo")

import numpy as np

# Model dims (hardcoded per problem spec)
H, KVH, D, HID = 32, 8, 128, 4096
S = 2048
THETA = 10000.0
NCORES = 8
QH = H // NCORES          # 4 query heads per core
P = 128                   # partitions
SC = 512                  # sequence chunk (matmul free dim)
NS = S // SC              # 4 chunks
KT = HID // P             # 32 contraction tiles for the projections
ST = S // P               # 16 sequence tiles of 128
NQK = QH + 2              # col-tiles per core in wqkv: q0..q3, k, v
WCOLS = NQK * P           # 768
GK = 4                    # weight k-tiles per DMA group
NG = KT // GK             # 8 groups
ISQRT_D = float(D) ** -0.5

_CACHE = {}


def _build():
    import concourse.bass as bass
    import concourse.tile as tile
    from concourse import bacc, mybir
    from contextlib import ExitStack

    F32 = mybir.dt.float32
    F32R = mybir.dt.float32r
    F16 = mybir.dt.float16
    AF = mybir.ActivationFunctionType

    nc = bacc.Bacc(
        "TRN2",
        target_bir_lowering=False,
        debug=False,
        enable_asserts=True,
        num_devices=NCORES,
    )

    hT = nc.dram_tensor("hT", [HID, S], F16, kind="ExternalInput").ap()
    wqkv = nc.dram_tensor("wqkv", [HID, WCOLS], F16, kind="ExternalInput").ap()
    wo = nc.dram_tensor("wo", [QH * D, HID], F16, kind="ExternalInput").ap()
    cos2 = nc.dram_tensor("cos2", [P, S], F16, kind="ExternalInput").ap()
    sinn2 = nc.dram_tensor("sinn2", [P, S], F16, kind="ExternalInput").ap()
    maskd = nc.dram_tensor("maskd", [P, 4 * P], F16, kind="ExternalInput").ap()
    ident = nc.dram_tensor("ident", [P, P], F16, kind="ExternalInput").ap()
    onesd = nc.dram_tensor("onesd", [P, 1], F16, kind="ExternalInput").ap()
    out = nc.dram_tensor("out", [S // NCORES, HID], F16, kind="ExternalOutput").ap()
    # per-chunk partial / rs tensors: separate DRAM tensors so the
    # whole-tensor WAR tracking never serializes chunk c+1's o_proj DMA
    # writes behind chunk c's in-flight ReduceScatter. The last chunk is
    # split in half to shorten the exposed collective tail.
    partials = [
        nc.dram_tensor(f"partial{c}", [SC, HID], F16).ap() for c in range(NS - 1)
    ]
    partials += [
        nc.dram_tensor("partial3a", [SC // 2, HID], F16).ap(),
        nc.dram_tensor("partial3b", [SC // 2, HID], F16).ap(),
    ]
    rs_outs = [
        nc.dram_tensor(f"rs{c}", [64, HID], F16).ap() for c in range(NS - 1)
    ]
    rs_outs += [
        nc.dram_tensor("rs3a", [32, HID], F16).ap(),
        nc.dram_tensor("rs3b", [32, HID], F16).ap(),
    ]

    groups = [list(range(NCORES))]

    with tile.TileContext(nc) as tc:
        with ExitStack() as ctx:
            # ---------------- constants (whole-kernel lifetime) ----------------
            cpool = ctx.enter_context(tc.tile_pool(name="const", bufs=1))
            mask_t = cpool.tile([P, 4 * P], F16, name="mask_t")
            ident_t = cpool.tile([P, P], F16, name="ident_t")
            ones_t = cpool.tile([P, 1], F16, name="ones_t")
            cos_t = cpool.tile([P, S], F16, name="cos_t")
            sin_t = cpool.tile([P, S], F16, name="sin_t")

            # ------------- persistent activation buffers -------------
            ppool = ctx.enter_context(tc.tile_pool(name="persist", bufs=1))
            qk = {}
            for n in range(QH + 1):
                for c in range(NS):
                    qk[(n, c)] = ppool.tile(
                        [P, SC], F16, name=f"qk{n}_{c}", tag=f"qk{n}_{c}"
                    )
            vT = [
                ppool.tile([P, SC], F16, name=f"vT{c}", tag=f"vT{c}")
                for c in range(NS)
            ]
            vnat = [
                ppool.tile([P, P], F16, name=f"vn{t}", tag=f"vn{t}")
                for t in range(ST)
            ]
            attnT = {}
            for h in range(QH):
                for c in range(NS):
                    attnT[(h, c)] = ppool.tile(
                        [P, SC], F16, name=f"at{h}_{c}", tag=f"at{h}_{c}"
                    )

            # ------------- resident weights (wqkv + wo, fp16) -------------
            # wqkv groups load first (phase A needs them immediately); the
            # small constants follow; wo loads are deferred until after
            # phase A emission so they don't delay the QKV pipeline.
            wpool = ctx.enter_context(tc.tile_pool(name="wres", bufs=1))
            wo_t = [
                wpool.tile([P, HID], F16, name=f"wo{hh}", tag=f"wo{hh}")
                for hh in range(QH)
            ]
            wq_g = {}

            def load_group(g):
                wq_g[g] = wpool.tile(
                    [P, GK * WCOLS], F16, name=f"wqg{g}", tag=f"wqg{g}"
                )
                src = wqkv[g * GK * P : (g + 1) * GK * P, :].rearrange(
                    "(t p) n -> p t n", p=P
                )
                dst = wq_g[g][:].rearrange("p (t n) -> p t n", t=GK)
                nc.sync.dma_start(dst, src)

            def wslice(k, n):
                g, kk = divmod(k, GK)
                off = kk * WCOLS + n * P
                return wq_g[g][:, off : off + P]

            for g in range(NG):
                load_group(g)
            nc.sync.dma_start(mask_t[:], maskd)
            nc.sync.dma_start(ident_t[:], ident)
            nc.sync.dma_start(ones_t[:], onesd)
            nc.sync.dma_start(cos_t[:], cos2)
            nc.sync.dma_start(sin_t[:], sinn2)

            # ---------------- stage A: QKV projection + RoPE + vT ------------
            half = P // 2
            with tc.tile_pool(name="ht", bufs=4) as h_pool, tc.tile_pool(
                name="psA", bufs=NQK, space="PSUM"
            ) as psA, tc.tile_pool(
                name="pst", bufs=2, space="PSUM"
            ) as pst, tc.tile_pool(name="ropet", bufs=3) as rpool:
                for c in range(NS):
                    ps = [
                        psA.tile([P, SC], F32, name=f"psA{n}", tag="psA")
                        for n in range(NQK)
                    ]
                    for k in range(KT):
                        ht_t = h_pool.tile([P, SC], F16, name="ht_t", tag="ht")
                        nc.sync.dma_start(
                            ht_t[:], hT[k * P : (k + 1) * P, c * SC : (c + 1) * SC]
                        )
                        for n in range(NQK):
                            nc.tensor.matmul(
                                ps[n][:],
                                wslice(k, n),
                                ht_t[:],
                                start=(k == 0),
                                stop=(k == KT - 1),
                            )
                    for n in range(QH + 1):
                        nc.scalar.copy(qk[(n, c)][:], ps[n][:])
                    nc.scalar.copy(vT[c][:], ps[NQK - 1][:])

                    # RoPE in place on this chunk's q heads and k (DVE),
                    # overlapped with the next chunk's QKV matmuls
                    csl = cos_t[:, c * SC : (c + 1) * SC]
                    ssl = sin_t[:, c * SC : (c + 1) * SC]
                    for n in range(QH + 1):
                        src = qk[(n, c)]
                        swp = rpool.tile([P, SC], F16, name="swp", tag="swp")
                        t1 = rpool.tile([P, SC], F16, name="t1", tag="t1")
                        nc.sync.dma_start(swp[0:half, :], src[half:P, :])
                        nc.sync.dma_start(swp[half:P, :], src[0:half, :])
                        nc.vector.tensor_mul(t1[:], src[:], csl)
                        nc.vector.tensor_mul(swp[:], swp[:], ssl)
                        nc.vector.tensor_add(src[:], t1[:], swp[:])

                # v transpose to natural [s, d]
                for t in range(ST):
                    c, j = divmod(t, NS)
                    tp = pst.tile([P, P], F16, name="tp", tag="tp")
                    nc.tensor.transpose(
                        tp[:], vT[c][:, j * P : (j + 1) * P], ident_t[:]
                    )
                    nc.scalar.copy(vnat[t][:], tp[:])

            # wo loads queue behind phase A's DMA stream (needed ~200us in)
            for hh in range(QH):
                nc.sync.dma_start(wo_t[hh][:], wo[hh * P : (hh + 1) * P, :])

            # ---------- attention + o_proj + reduce-scatter ----------
            with tc.tile_pool(name="pssc", bufs=2, space="PSUM") as ps_sc, tc.tile_pool(
                name="pssm", bufs=2, space="PSUM"
            ) as ps_sm, tc.tile_pool(
                name="pspv", bufs=2, space="PSUM"
            ) as ps_pv, tc.tile_pool(
                name="psop", bufs=2, space="PSUM"
            ) as ps_op, tc.tile_pool(name="expp", bufs=6) as ep, tc.tile_pool(
                name="esump", bufs=2
            ) as esp, tc.tile_pool(name="smallp", bufs=2) as sp, tc.tile_pool(
                name="stagep", bufs=32
            ) as stp:
                # stagep is deep on purpose: while a ReduceScatter is in
                # flight the SDMA engines starve regular DMA queues, so a
                # full chunk of partial-write DMAs (32 tiles) must be able
                # to back up without blocking the eviction engines.

                def emit_oproj_group(c, jj, nn, eng):
                    op = ps_op.tile([P, SC], F32, name="op", tag="op")
                    for h in range(QH):
                        nc.tensor.matmul(
                            op[:],
                            attnT[(h, c)][:, jj * P : (jj + 1) * P],
                            wo_t[h][:, nn * SC : (nn + 1) * SC],
                            start=(h == 0),
                            stop=(h == QH - 1),
                        )
                    st = stp.tile([P, SC], F16, name="st", tag="st")
                    # alternate eviction between Act and DVE to balance load
                    if eng == 0:
                        nc.scalar.copy(st[:], op[:])
                    else:
                        nc.vector.tensor_copy(st[:], op[:])
                    if c < NS - 1:
                        dst = partials[c]
                        row = jj * P
                    else:
                        dst = partials[NS - 1 + jj // 2]
                        row = (jj % 2) * P
                    nc.sync.dma_start(
                        dst[row : row + P, nn * SC : (nn + 1) * SC], st[:]
                    )

                def oproj_list(c):
                    return [(c, jj, nn) for jj in range(QH) for nn in range(HID // SC)]

                def emit_rs(idx):
                    # idx 0..2: full chunks; idx 3,4: halves of chunk 3
                    rows = 64 if idx < NS - 1 else 32
                    off = idx * 64 if idx < NS - 1 else 192 + (idx - NS + 1) * 32
                    nc.gpsimd.collective_compute(
                        "ReduceScatter",
                        mybir.AluOpType.add,
                        replica_groups=groups,
                        ins=[partials[idx][:, :]],
                        outs=[rs_outs[idx][:, :]],
                    )
                    nc.sync.dma_start(out[off : off + rows, :], rs_outs[idx][:, :])

                for c in range(NS):
                    prev = oproj_list(c - 1) if c > 0 else []
                    nsk = QH * c + QH  # causal: sk tiles for this chunk
                    total_steps = QH * nsk
                    oi = 0
                    si = 0
                    for h in range(QH):
                        esum = esp.tile([P, SC], F16, name="esum", tag="esum")
                        pv = ps_pv.tile([P, SC], F32, name="pv", tag="pv")
                        qrhs = qk[(h, c)][:]
                        for t in range(nsk):
                            kc, kj = divmod(t, NS)
                            ktile = qk[(QH, kc)][:, kj * P : (kj + 1) * P]
                            # diagonal tiles: columns below j*P are fully
                            # masked; compute only the live region
                            lo = (t - QH * c) * P if t >= QH * c else 0
                            sc_ps = ps_sc.tile([P, SC], F32, name="sc_ps", tag="sc")
                            nc.tensor.matmul(
                                sc_ps[:, lo:SC], ktile, qrhs[:, lo:SC],
                                start=True, stop=True,
                            )
                            e = ep.tile([P, SC], F16, name="e", tag="e")
                            nc.scalar.activation(
                                e[:, lo:SC], sc_ps[:, lo:SC], AF.Exp, scale=ISQRT_D
                            )
                            if t >= QH * c:
                                # triangular mask on the diagonal P-block
                                nc.vector.tensor_mul(
                                    e[:, lo : lo + P],
                                    e[:, lo : lo + P],
                                    mask_t[:, 3 * P : 4 * P],
                                )
                            if t == 0:
                                nc.vector.tensor_copy(esum[:], e[:])
                            else:
                                nc.vector.tensor_add(
                                    esum[:, lo:SC], esum[:, lo:SC], e[:, lo:SC]
                                )
                            nc.tensor.matmul(
                                pv[:, lo:SC], vnat[t][:], e[:, lo:SC],
                                start=(t == 0), stop=(t == nsk - 1),
                            )
                            si += 1
                            # interleave the previous chunk's o_proj: start
                            # a few steps in (its attnT normalization chain
                            # is still completing), finish by 3/4 of the
                            # loop so its ReduceScatter fires early
                            start_si = 6
                            end_si = max(start_si + 1, (3 * total_steps) // 4)
                            frac = (si - start_si) / (end_si - start_si)
                            want = int(len(prev) * min(max(frac, 0.0), 1.0))
                            while prev and oi < want:
                                cc, jj, nn = prev[oi]
                                emit_oproj_group(cc, jj, nn, oi % 2)
                                oi += 1
                        sm = ps_sm.tile([1, SC], F32, name="sm", tag="sm")
                        nc.tensor.matmul(
                            sm[:], ones_t[:], esum[:], start=True, stop=True
                        )
                        # iterative-divide reciprocal is ~6 cyc/elem; the
                        # approx variant (~51 ULP) is 5x faster and far
                        # inside the 2e-2 budget. Reciprocate the [1,512]
                        # row, then broadcast the result.
                        smh = sp.tile([1, SC], F32, name="smh", tag="smh")
                        rcp = sp.tile([1, SC], F32, name="rcp", tag="rcp")
                        bc = sp.tile([P, SC], F32, name="bc", tag="bc")
                        nc.scalar.copy(smh[:], sm[:])
                        nc.vector.reciprocal_approx_fast(rcp[:], smh[:])
                        nc.gpsimd.partition_broadcast(bc[:], rcp[:])
                        nc.vector.tensor_mul(attnT[(h, c)][:], pv[:], bc[:])
                    while oi < len(prev):
                        cc, jj, nn = prev[oi]
                        emit_oproj_group(cc, jj, nn, oi % 2)
                        oi += 1
                    if c > 0:
                        emit_rs(c - 1)

                last = oproj_list(NS - 1)
                for idx, (cc, jj, nn) in enumerate(last):
                    emit_oproj_group(cc, jj, nn, idx % 2)
                    if idx == len(last) // 2 - 1:
                        emit_rs(NS - 1)  # first half of chunk 3
                emit_rs(NS)  # second half of chunk 3

    nc.compile()
    return nc


def _get_nc():
    if "nc" not in _CACHE:
        _CACHE["nc"] = _build()
    return _CACHE["nc"]


def _host_inputs(positions, hidden_states, Wqkv, Wo):
    """Shard + relayout the full inputs for the 8 cores (fp16 device side)."""
    pos = np.asarray(positions).reshape(-1).astype(np.float64)  # [S]
    hs = np.asarray(hidden_states, dtype=np.float32).reshape(S, HID)
    Wqkv = np.asarray(Wqkv, dtype=np.float32)
    Wo = np.asarray(Wo, dtype=np.float32)

    hT = np.ascontiguousarray(hs.T).astype(np.float16)  # [HID, S]

    half = D // 2
    inv_freq = 1.0 / (THETA ** (np.arange(half, dtype=np.float64) / half))
    ang = pos[None, :] * inv_freq[:, None]  # [64, S]
    cos = np.cos(ang)
    sin = np.sin(ang)
    cos2 = np.ascontiguousarray(np.concatenate([cos, cos], axis=0)).astype(
        np.float16
    )
    sinn2 = np.ascontiguousarray(np.concatenate([-sin, sin], axis=0)).astype(
        np.float16
    )

    # causal mask, [sk, sq] orientation: [zeros(128x384) | upper-tri(128x128)].
    maskd = np.concatenate(
        [np.zeros((P, 3 * P), dtype=np.float16),
         np.triu(np.ones((P, P), dtype=np.float16))], axis=1)
    ident = np.eye(P, dtype=np.float16)
    onesd = np.ones((P, 1), dtype=np.float16)

    qb = Wqkv[:, : H * D]
    kb = Wqkv[:, H * D : H * D + KVH * D]
    vb = Wqkv[:, H * D + KVH * D :]

    in_maps = []
    for c in range(NCORES):
        wq_c = np.concatenate(
            [
                qb[:, c * QH * D : (c + 1) * QH * D],
                kb[:, c * D : (c + 1) * D],
                vb[:, c * D : (c + 1) * D],
            ],
            axis=1,
        ).astype(np.float16)
        wo_c = Wo[c * QH * D : (c + 1) * QH * D, :].astype(np.float16)
        in_maps.append(
            {
                "hT": hT,
                "wqkv": np.ascontiguousarray(wq_c),
                "wo": np.ascontiguousarray(wo_c),
                "cos2": cos2,
                "sinn2": sinn2,
                "maskd": maskd,
                "ident": ident,
                "onesd": onesd,
            }
        )
    return in_maps


def _assemble(results):
    full = np.empty((S, HID), dtype=np.float32)
    for c in range(NCORES):
        oc = np.asarray(results[c]["out"], dtype=np.float32)  # [256, HID]
        for j in range(NS - 1):
            full[SC * j + 64 * c : SC * j + 64 * (c + 1), :] = oc[
                64 * j : 64 * (j + 1), :
            ]
        # chunk 3 was reduce-scattered as two 256-row halves
        full[3 * SC + 32 * c : 3 * SC + 32 * (c + 1), :] = oc[192:224, :]
        full[3 * SC + 256 + 32 * c : 3 * SC + 256 + 32 * (c + 1), :] = oc[
            224:256, :
        ]
    return full.reshape(1, S, HID)


def kernel(positions, hidden_states, Wqkv, Wo):
    from concourse.bass_utils import run_bass_kernel_spmd

    nc = _get_nc()
    in_maps = _host_inputs(positions, hidden_states, Wqkv, Wo)
    res = run_bass_kernel_spmd(nc, in_maps, core_ids=list(range(NCORES)))
    return _assemble(res.results)


def kernel_timed(positions, hidden_states, Wqkv, Wo, tmpdir="/tmp/ntff_trace"):
    """Like kernel() but with NTFF profiling; returns (output, exec_time_ns)."""
    import os
    import shutil

    from concourse.bass_utils import run_bass_kernel_spmd

    shutil.rmtree(tmpdir, ignore_errors=True)
    os.makedirs(tmpdir, exist_ok=True)
    nc = _get_nc()
    in_maps = _host_inputs(positions, hidden_states, Wqkv, Wo)
    res = run_bass_kernel_spmd(
        nc, in_maps, core_ids=list(range(NCORES)), trace=True, tmpdir=tmpdir
    )
    return _assemble(res.results), res.exec_time_ns


# revision 26
# speedup vs baseline: 1.1744x; 1.0019x over previous
"""Llama attention layer (B=1, S=2048, H=32, KVH=8, D=128, HID=4096) on 8 TRN2
NeuronCores.

Sharding: tensor-parallel over head groups. Core c computes Q heads
[4c..4c+4) and KV head c end-to-end (QKV projection, RoPE, causal GQA
attention, o_proj rows for its heads), then a chunked ReduceScatter sums the
o_proj partials so core c ends up with rows {512j + 64c .. 512j + 64c + 64}
of the output for j in 0..3. The host reassembles the full [2048, 4096]
output by concatenating the shards.

v2 design (fp16 end-to-end, pipelined):
  - All DRAM-resident tensors are fp16 (half the HBM traffic and half the
    collective bytes of fp32; fp16 matmuls run at full PE rate and carry
    10-bit mantissas). PSUM accumulation stays fp32.
  - Single-pass QKV: the whole wqkv shard (6.3 MB fp16) is SBUF-resident,
    hT streams through once. RoPE runs on DVE per chunk, overlapped with
    the next chunk's QKV matmuls.
  - Softmax denominators accumulate on DVE (esum += exp tile) instead of
    per-tile PE ones-matmuls; one [1,512] ones-matmul per (chunk, head)
    reduces esum across partitions.
  - o_proj matmul groups of chunk c-1 are interleaved into the attention
    t-loop of chunk c so the PE never idles waiting on the scalar engine's
    exp tiles.
  - Per-chunk fp16 ReduceScatter overlaps the remaining compute.
"""

import sys

if "/opt/trn_rl_repo" not in sys.path:
    sys.path.insert(0, "/opt/trn_rl_repo")

import numpy as np

# Model dims (hardcoded per problem spec)
H, KVH, D, HID = 32, 8, 128, 4096
S = 2048
THETA = 10000.0
NCORES = 8
QH = H // NCORES          # 4 query heads per core
P = 128                   # partitions
SC = 512                  # sequence chunk (matmul free dim)
NS = S // SC              # 4 chunks
KT = HID // P             # 32 contraction tiles for the projections
ST = S // P               # 16 sequence tiles of 128
NQK = QH + 2              # col-tiles per core in wqkv: q0..q3, k, v
WCOLS = NQK * P           # 768
GK = 4                    # weight k-tiles per DMA group
NG = KT // GK             # 8 groups
ISQRT_D = float(D) ** -0.5

_CACHE = {}


def _build():
    import concourse.bass as bass
    import concourse.tile as tile
    from concourse import bacc, mybir
    from contextlib import ExitStack

    F32 = mybir.dt.float32
    F32R = mybir.dt.float32r
    F16 = mybir.dt.float16
    AF = mybir.ActivationFunctionType

    nc = bacc.Bacc(
        "TRN2",
        target_bir_lowering=False,
        debug=False,
        enable_asserts=True,
        num_devices=NCORES,
    )

    hT = nc.dram_tensor("hT", [HID, S], F16, kind="ExternalInput").ap()
    wqkv = nc.dram_tensor("wqkv", [HID, WCOLS], F16, kind="ExternalInput").ap()
    wo = nc.dram_tensor("wo", [QH * D, HID], F16, kind="ExternalInput").ap()
    cos2 = nc.dram_tensor("cos2", [P, S], F16, kind="ExternalInput").ap()
    sinn2 = nc.dram_tensor("sinn2", [P, S], F16, kind="ExternalInput").ap()
    maskd = nc.dram_tensor("maskd", [P, 4 * P], F16, kind="ExternalInput").ap()
    ident = nc.dram_tensor("ident", [P, P], F16, kind="ExternalInput").ap()
    onesd = nc.dram_tensor("onesd", [P, 1], F16, kind="ExternalInput").ap()
    out = nc.dram_tensor("out", [S // NCORES, HID], F16, kind="ExternalOutput").ap()
    # per-chunk partial / rs tensors: separate DRAM tensors so the
    # whole-tensor WAR tracking never serializes chunk c+1's o_proj DMA
    # writes behind chunk c's in-flight ReduceScatter. The last chunk is
    # split in half to shorten the exposed collective tail.
    partials = [
        nc.dram_tensor(f"partial{c}", [SC, HID], F16).ap() for c in range(NS - 1)
    ]
    partials += [
        nc.dram_tensor("partial3a", [SC // 2, HID], F16).ap(),
        nc.dram_tensor("partial3b", [SC // 2, HID], F16).ap(),
    ]
    rs_outs = [
        nc.dram_tensor(f"rs{c}", [64, HID], F16).ap() for c in range(NS - 1)
    ]
    rs_outs += [
        nc.dram_tensor("rs3a", [32, HID], F16).ap(),
        nc.dram_tensor("rs3b", [32, HID], F16).ap(),
    ]
    # tiny dummy collective: absorbs the first-call warmup (~15us slower
    # algbw) during phase A instead of on the critical RS0
    dum_in = nc.dram_tensor("dum_in", [NCORES, 512], F16).ap()
    dum_out = nc.dram_tensor("dum_out", [1, 512], F16).ap()

    groups = [list(range(NCORES))]

    with tile.TileContext(nc) as tc:
        with ExitStack() as ctx:
            # ---------------- constants (whole-kernel lifetime) ----------------
            cpool = ctx.enter_context(tc.tile_pool(name="const", bufs=1))
            mask_t = cpool.tile([P, 4 * P], F16, name="mask_t")
            ident_t = cpool.tile([P, P], F16, name="ident_t")
            ones_t = cpool.tile([P, 1], F16, name="ones_t")
            cos_t = cpool.tile([P, S], F16, name="cos_t")
            sin_t = cpool.tile([P, S], F16, name="sin_t")

            # ------------- persistent activation buffers -------------
            ppool = ctx.enter_context(tc.tile_pool(name="persist", bufs=1))
            qk = {}
            for n in range(QH + 1):
                for c in range(NS):
                    qk[(n, c)] = ppool.tile(
                        [P, SC], F16, name=f"qk{n}_{c}", tag=f"qk{n}_{c}"
                    )
            vT = [
                ppool.tile([P, SC], F16, name=f"vT{c}", tag=f"vT{c}")
                for c in range(NS)
            ]
            vnat = [
                ppool.tile([P, P], F16, name=f"vn{t}", tag=f"vn{t}")
                for t in range(ST)
            ]
            attnT = {}
            for h in range(QH):
                for c in range(NS):
                    attnT[(h, c)] = ppool.tile(
                        [P, SC], F16, name=f"at{h}_{c}", tag=f"at{h}_{c}"
                    )

            # ------------- resident weights (wqkv + wo, fp16) -------------
            # wqkv groups load first (phase A needs them immediately); the
            # small constants follow; wo loads are deferred until after
            # phase A emission so they don't delay the QKV pipeline.
            wpool = ctx.enter_context(tc.tile_pool(name="wres", bufs=1))
            wo_t = [
                wpool.tile([P, HID], F16, name=f"wo{hh}", tag=f"wo{hh}")
                for hh in range(QH)
            ]
            wq_g = {}

            def load_group(g):
                wq_g[g] = wpool.tile(
                    [P, GK * WCOLS], F16, name=f"wqg{g}", tag=f"wqg{g}"
                )
                src = wqkv[g * GK * P : (g + 1) * GK * P, :].rearrange(
                    "(t p) n -> p t n", p=P
                )
                dst = wq_g[g][:].rearrange("p (t n) -> p t n", t=GK)
                nc.sync.dma_start(dst, src)

            def wslice(k, n):
                g, kk = divmod(k, GK)
                off = kk * WCOLS + n * P
                return wq_g[g][:, off : off + P]

            for g in range(NG):
                load_group(g)
            nc.gpsimd.collective_compute(
                "ReduceScatter",
                mybir.AluOpType.add,
                replica_groups=groups,
                ins=[dum_in[:, :]],
                outs=[dum_out[:, :]],
            )
            nc.sync.dma_start(mask_t[:], maskd)
            nc.sync.dma_start(ident_t[:], ident)
            nc.sync.dma_start(ones_t[:], onesd)
            nc.sync.dma_start(cos_t[:], cos2)
            nc.sync.dma_start(sin_t[:], sinn2)

            # ---------------- stage A: QKV projection + RoPE + vT ------------
            half = P // 2
            with tc.tile_pool(name="ht", bufs=4) as h_pool, tc.tile_pool(
                name="psA", bufs=NQK, space="PSUM"
            ) as psA, tc.tile_pool(
                name="pst", bufs=2, space="PSUM"
            ) as pst, tc.tile_pool(name="ropet", bufs=3) as rpool:
                for c in range(NS):
                    ps = [
                        psA.tile([P, SC], F32, name=f"psA{n}", tag="psA")
                        for n in range(NQK)
                    ]
                    for k in range(KT):
                        ht_t = h_pool.tile([P, SC], F16, name="ht_t", tag="ht")
                        nc.sync.dma_start(
                            ht_t[:], hT[k * P : (k + 1) * P, c * SC : (c + 1) * SC]
                        )
                        for n in range(NQK):
                            nc.tensor.matmul(
                                ps[n][:],
                                wslice(k, n),
                                ht_t[:],
                                start=(k == 0),
                                stop=(k == KT - 1),
                            )
                    for n in range(QH + 1):
                        nc.scalar.copy(qk[(n, c)][:], ps[n][:])
                    nc.scalar.copy(vT[c][:], ps[NQK - 1][:])

                    # RoPE in place on this chunk's q heads and k (DVE),
                    # overlapped with the next chunk's QKV matmuls
                    csl = cos_t[:, c * SC : (c + 1) * SC]
                    ssl = sin_t[:, c * SC : (c + 1) * SC]
                    for n in range(QH + 1):
                        src = qk[(n, c)]
                        swp = rpool.tile([P, SC], F16, name="swp", tag="swp")
                        t1 = rpool.tile([P, SC], F16, name="t1", tag="t1")
                        nc.sync.dma_start(swp[0:half, :], src[half:P, :])
                        nc.sync.dma_start(swp[half:P, :], src[0:half, :])
                        nc.vector.tensor_mul(t1[:], src[:], csl)
                        nc.vector.tensor_mul(swp[:], swp[:], ssl)
                        nc.vector.tensor_add(src[:], t1[:], swp[:])

                # v transpose to natural [s, d]
                for t in range(ST):
                    c, j = divmod(t, NS)
                    tp = pst.tile([P, P], F16, name="tp", tag="tp")
                    nc.tensor.transpose(
                        tp[:], vT[c][:, j * P : (j + 1) * P], ident_t[:]
                    )
                    nc.scalar.copy(vnat[t][:], tp[:])

            # wo loads queue behind phase A's DMA stream (needed ~200us in)
            for hh in range(QH):
                nc.sync.dma_start(wo_t[hh][:], wo[hh * P : (hh + 1) * P, :])

            # ---------- attention + o_proj + reduce-scatter ----------
            with tc.tile_pool(name="pssc", bufs=2, space="PSUM") as ps_sc, tc.tile_pool(
                name="pssm", bufs=2, space="PSUM"
            ) as ps_sm, tc.tile_pool(
                name="pspv", bufs=2, space="PSUM"
            ) as ps_pv, tc.tile_pool(
                name="psop", bufs=2, space="PSUM"
            ) as ps_op, tc.tile_pool(name="expp", bufs=6) as ep, tc.tile_pool(
                name="esump", bufs=2
            ) as esp, tc.tile_pool(name="smallp", bufs=2) as sp, tc.tile_pool(
                name="stagep", bufs=32
            ) as stp:
                # stagep is deep on purpose: while a ReduceScatter is in
                # flight the SDMA engines starve regular DMA queues, so a
                # full chunk of partial-write DMAs (32 tiles) must be able
                # to back up without blocking the eviction engines.

                def emit_oproj_group(c, jj, nn, eng):
                    op = ps_op.tile([P, SC], F32, name="op", tag="op")
                    for h in range(QH):
                        nc.tensor.matmul(
                            op[:],
                            attnT[(h, c)][:, jj * P : (jj + 1) * P],
                            wo_t[h][:, nn * SC : (nn + 1) * SC],
                            start=(h == 0),
                            stop=(h == QH - 1),
                        )
                    st = stp.tile([P, SC], F16, name="st", tag="st")
                    # alternate eviction between Act and DVE to balance load
                    if eng == 0:
                        nc.scalar.copy(st[:], op[:])
                    else:
                        nc.vector.tensor_copy(st[:], op[:])
                    if c < NS - 1:
                        dst = partials[c]
                        row = jj * P
                    else:
                        dst = partials[NS - 1 + jj // 2]
                        row = (jj % 2) * P
                    nc.sync.dma_start(
                        dst[row : row + P, nn * SC : (nn + 1) * SC], st[:]
                    )

                def oproj_list(c):
                    return [(c, jj, nn) for jj in range(QH) for nn in range(HID // SC)]

                def emit_rs(idx):
                    # idx 0..2: full chunks; idx 3,4: halves of chunk 3
                    rows = 64 if idx < NS - 1 else 32
                    off = idx * 64 if idx < NS - 1 else 192 + (idx - NS + 1) * 32
                    nc.gpsimd.collective_compute(
                        "ReduceScatter",
                        mybir.AluOpType.add,
                        replica_groups=groups,
                        ins=[partials[idx][:, :]],
                        outs=[rs_outs[idx][:, :]],
                    )
                    nc.sync.dma_start(out[off : off + rows, :], rs_outs[idx][:, :])

                for c in range(NS):
                    prev = oproj_list(c - 1) if c > 0 else []
                    nsk = QH * c + QH  # causal: sk tiles for this chunk
                    total_steps = QH * nsk
                    oi = 0
                    si = 0
                    for h in range(QH):
                        esum = esp.tile([P, SC], F16, name="esum", tag="esum")
                        pv = ps_pv.tile([P, SC], F32, name="pv", tag="pv")
                        qrhs = qk[(h, c)][:]
                        for t in range(nsk):
                            kc, kj = divmod(t, NS)
                            ktile = qk[(QH, kc)][:, kj * P : (kj + 1) * P]
                            # diagonal tiles: columns below j*P are fully
                            # masked; compute only the live region
                            lo = (t - QH * c) * P if t >= QH * c else 0
                            sc_ps = ps_sc.tile([P, SC], F32, name="sc_ps", tag="sc")
                            nc.tensor.matmul(
                                sc_ps[:, lo:SC], ktile, qrhs[:, lo:SC],
                                start=True, stop=True,
                            )
                            e = ep.tile([P, SC], F16, name="e", tag="e")
                            nc.scalar.activation(
                                e[:, lo:SC], sc_ps[:, lo:SC], AF.Exp, scale=ISQRT_D
                            )
                            if t >= QH * c:
                                # triangular mask on the diagonal P-block
                                nc.vector.tensor_mul(
                                    e[:, lo : lo + P],
                                    e[:, lo : lo + P],
                                    mask_t[:, 3 * P : 4 * P],
                                )
                            if t == 0:
                                nc.vector.tensor_copy(esum[:], e[:])
                            else:
                                nc.vector.tensor_add(
                                    esum[:, lo:SC], esum[:, lo:SC], e[:, lo:SC]
                                )
                            nc.tensor.matmul(
                                pv[:, lo:SC], vnat[t][:], e[:, lo:SC],
                                start=(t == 0), stop=(t == nsk - 1),
                            )
                            si += 1
                            # interleave the previous chunk's o_proj: start
                            # a few steps in (its attnT normalization chain
                            # is still completing), finish by 3/4 of the
                            # loop so its ReduceScatter fires early
                            start_si = 6
                            end_si = max(start_si + 1, total_steps // 2)
                            frac = (si - start_si) / (end_si - start_si)
                            want = int(len(prev) * min(max(frac, 0.0), 1.0))
                            while prev and oi < want:
                                cc, jj, nn = prev[oi]
                                emit_oproj_group(cc, jj, nn, oi % 2)
                                oi += 1
                        sm = ps_sm.tile([1, SC], F32, name="sm", tag="sm")
                        nc.tensor.matmul(
                            sm[:], ones_t[:], esum[:], start=True, stop=True
                        )
                        # iterative-divide reciprocal is ~6 cyc/elem; the
                        # approx variant (~51 ULP) is 5x faster and far
                        # inside the 2e-2 budget. Reciprocate the [1,512]
                        # row, then broadcast the result.
                        smh = sp.tile([1, SC], F32, name="smh", tag="smh")
                        rcp = sp.tile([1, SC], F32, name="rcp", tag="rcp")
                        bc = sp.tile([P, SC], F32, name="bc", tag="bc")
                        nc.scalar.copy(smh[:], sm[:])
                        nc.vector.reciprocal_approx_fast(rcp[:], smh[:])
                        nc.gpsimd.partition_broadcast(bc[:], rcp[:])
                        nc.vector.tensor_mul(attnT[(h, c)][:], pv[:], bc[:])
                    while oi < len(prev):
                        cc, jj, nn = prev[oi]
                        emit_oproj_group(cc, jj, nn, oi % 2)
                        oi += 1
                    if c > 0:
                        emit_rs(c - 1)

                last = oproj_list(NS - 1)
                for idx, (cc, jj, nn) in enumerate(last):
                    emit_oproj_group(cc, jj, nn, idx % 2)
                    if idx == len(last) // 2 - 1:
                        emit_rs(NS - 1)  # first half of chunk 3
                emit_rs(NS)  # second half of chunk 3

    nc.compile()
    return nc


def _get_nc():
    if "nc" not in _CACHE:
        _CACHE["nc"] = _build()
    return _CACHE["nc"]


def _host_inputs(positions, hidden_states, Wqkv, Wo):
    """Shard + relayout the full inputs for the 8 cores (fp16 device side)."""
    pos = np.asarray(positions).reshape(-1).astype(np.float64)  # [S]
    hs = np.asarray(hidden_states, dtype=np.float32).reshape(S, HID)
    Wqkv = np.asarray(Wqkv, dtype=np.float32)
    Wo = np.asarray(Wo, dtype=np.float32)

    hT = np.ascontiguousarray(hs.T).astype(np.float16)  # [HID, S]

    half = D // 2
    inv_freq = 1.0 / (THETA ** (np.arange(half, dtype=np.float64) / half))
    ang = pos[None, :] * inv_freq[:, None]  # [64, S]
    cos = np.cos(ang)
    sin = np.sin(ang)
    cos2 = np.ascontiguousarray(np.concatenate([cos, cos], axis=0)).astype(
        np.float16
    )
    sinn2 = np.ascontiguousarray(np.concatenate([-sin, sin], axis=0)).astype(
        np.float16
    )

    # causal mask, [sk, sq] orientation: [zeros(128x384) | upper-tri(128x128)].
    maskd = np.concatenate(
        [np.zeros((P, 3 * P), dtype=np.float16),
         np.triu(np.ones((P, P), dtype=np.float16))], axis=1)
    ident = np.eye(P, dtype=np.float16)
    onesd = np.ones((P, 1), dtype=np.float16)

    qb = Wqkv[:, : H * D]
    kb = Wqkv[:, H * D : H * D + KVH * D]
    vb = Wqkv[:, H * D + KVH * D :]

    in_maps = []
    for c in range(NCORES):
        wq_c = np.concatenate(
            [
                qb[:, c * QH * D : (c + 1) * QH * D],
                kb[:, c * D : (c + 1) * D],
                vb[:, c * D : (c + 1) * D],
            ],
            axis=1,
        ).astype(np.float16)
        wo_c = Wo[c * QH * D : (c + 1) * QH * D, :].astype(np.float16)
        in_maps.append(
            {
                "hT": hT,
                "wqkv": np.ascontiguousarray(wq_c),
                "wo": np.ascontiguousarray(wo_c),
                "cos2": cos2,
                "sinn2": sinn2,
                "maskd": maskd,
                "ident": ident,
                "onesd": onesd,
            }
        )
    return in_maps


def _assemble(results):
    full = np.empty((S, HID), dtype=np.float32)
    for c in range(NCORES):
        oc = np.asarray(results[c]["out"], dtype=np.float32)  # [256, HID]
        for j in range(NS - 1):
            full[SC * j + 64 * c : SC * j + 64 * (c + 1), :] = oc[
                64 * j : 64 * (j + 1), :
            ]
        # chunk 3 was reduce-scattered as two 256-row halves
        full[3 * SC + 32 * c : 3 * SC + 32 * (c + 1), :] = oc[192:224, :]
        full[3 * SC + 256 + 32 * c : 3 * SC + 256 + 32 * (c + 1), :] = oc[
            224:256, :
        ]
    return full.reshape(1, S, HID)


def kernel(positions, hidden_states, Wqkv, Wo):
    from concourse.bass_utils import run_bass_kernel_spmd

    nc = _get_nc()
    in_maps = _host_inputs(positions, hidden_states, Wqkv, Wo)
    res = run_bass_kernel_spmd(nc, in_maps, core_ids=list(range(NCORES)))
    return _assemble(res.results)


def kernel_timed(positions, hidden_states, Wqkv, Wo, tmpdir="/tmp/ntff_trace"):
    """Like kernel() but with NTFF profiling; returns (output, exec_time_ns)."""
    import os
    import shutil

    from concourse.bass_utils import run_bass_kernel_spmd

    shutil.rmtree(tmpdir, ignore_errors=True)
    os.makedirs(tmpdir, exist_ok=True)
    nc = _get_nc()
    in_maps = _host_inputs(positions, hidden_states, Wqkv, Wo)
    res = run_bass_kernel_spmd(
        nc, in_maps, core_ids=list(range(NCORES)), trace=True, tmpdir=tmpdir
    )
    return _assemble(res.results), res.exec_time_ns


# revision 27
# speedup vs baseline: 1.1910x; 1.0141x over previous
"""Llama attention layer (B=1, S=2048, H=32, KVH=8, D=128, HID=4096) on 8 TRN2
NeuronCores.

Sharding: tensor-parallel over head groups. Core c computes Q heads
[4c..4c+4) and KV head c end-to-end (QKV projection, RoPE, causal GQA
attention, o_proj rows for its heads), then a chunked ReduceScatter sums the
o_proj partials so core c ends up with rows {512j + 64c .. 512j + 64c + 64}
of the output for j in 0..3. The host reassembles the full [2048, 4096]
output by concatenating the shards.

v2 design (fp16 end-to-end, pipelined):
  - All DRAM-resident tensors are fp16 (half the HBM traffic and half the
    collective bytes of fp32; fp16 matmuls run at full PE rate and carry
    10-bit mantissas). PSUM accumulation stays fp32.
  - Single-pass QKV: the whole wqkv shard (6.3 MB fp16) is SBUF-resident,
    hT streams through once. RoPE runs on DVE per chunk, overlapped with
    the next chunk's QKV matmuls.
  - Softmax denominators accumulate on DVE (esum += exp tile) instead of
    per-tile PE ones-matmuls; one [1,512] ones-matmul per (chunk, head)
    reduces esum across partitions.
  - o_proj matmul groups of chunk c-1 are interleaved into the attention
    t-loop of chunk c so the PE never idles waiting on the scalar engine's
    exp tiles.
  - Per-chunk fp16 ReduceScatter overlaps the remaining compute.
"""

import sys

if "/opt/trn_rl_repo" not in sys.path:
    sys.path.insert(0, "/opt/trn_rl_repo")

import numpy as np

# Model dims (hardcoded per problem spec)
H, KVH, D, HID = 32, 8, 128, 4096
S = 2048
THETA = 10000.0
NCORES = 8
QH = H // NCORES          # 4 query heads per core
P = 128                   # partitions
SC = 512                  # sequence chunk (matmul free dim)
NS = S // SC              # 4 chunks
KT = HID // P             # 32 contraction tiles for the projections
ST = S // P               # 16 sequence tiles of 128
NQK = QH + 2              # col-tiles per core in wqkv: q0..q3, k, v
WCOLS = NQK * P           # 768
GK = 4                    # weight k-tiles per DMA group
NG = KT // GK             # 8 groups
ISQRT_D = float(D) ** -0.5

_CACHE = {}


def _build():
    import concourse.bass as bass
    import concourse.tile as tile
    from concourse import bacc, mybir
    from contextlib import ExitStack

    F32 = mybir.dt.float32
    F32R = mybir.dt.float32r
    F16 = mybir.dt.float16
    AF = mybir.ActivationFunctionType

    nc = bacc.Bacc(
        "TRN2",
        target_bir_lowering=False,
        debug=False,
        enable_asserts=True,
        num_devices=NCORES,
    )

    hT = nc.dram_tensor("hT", [HID, S], F16, kind="ExternalInput").ap()
    wqkv = nc.dram_tensor("wqkv", [HID, WCOLS], F16, kind="ExternalInput").ap()
    wo = nc.dram_tensor("wo", [QH * D, HID], F16, kind="ExternalInput").ap()
    cos2 = nc.dram_tensor("cos2", [P, S], F16, kind="ExternalInput").ap()
    sinn2 = nc.dram_tensor("sinn2", [P, S], F16, kind="ExternalInput").ap()
    maskd = nc.dram_tensor("maskd", [P, 4 * P], F16, kind="ExternalInput").ap()
    ident = nc.dram_tensor("ident", [P, P], F16, kind="ExternalInput").ap()
    onesd = nc.dram_tensor("onesd", [P, 1], F16, kind="ExternalInput").ap()
    out = nc.dram_tensor("out", [S // NCORES, HID], F16, kind="ExternalOutput").ap()
    # per-chunk partial / rs tensors: separate DRAM tensors so the
    # whole-tensor WAR tracking never serializes chunk c+1's o_proj DMA
    # writes behind chunk c's in-flight ReduceScatter. The last chunk is
    # split in half to shorten the exposed collective tail.
    partials = [
        nc.dram_tensor(f"partial{c}", [SC, HID], F16).ap() for c in range(NS - 1)
    ]
    partials += [
        nc.dram_tensor("partial3a", [SC // 2, HID], F16).ap(),
        nc.dram_tensor("partial3b", [SC // 2, HID], F16).ap(),
    ]
    rs_outs = [
        nc.dram_tensor(f"rs{c}", [64, HID], F16).ap() for c in range(NS - 1)
    ]
    rs_outs += [
        nc.dram_tensor("rs3a", [32, HID], F16).ap(),
        nc.dram_tensor("rs3b", [32, HID], F16).ap(),
    ]
    # tiny dummy collective: absorbs the first-call warmup (~15us slower
    # algbw) during phase A instead of on the critical RS0
    dum_in = nc.dram_tensor("dum_in", [NCORES, 512], F16).ap()
    dum_out = nc.dram_tensor("dum_out", [1, 512], F16).ap()

    groups = [list(range(NCORES))]

    with tile.TileContext(nc) as tc:
        with ExitStack() as ctx:
            # ---------------- constants (whole-kernel lifetime) ----------------
            cpool = ctx.enter_context(tc.tile_pool(name="const", bufs=1))
            mask_t = cpool.tile([P, 4 * P], F16, name="mask_t")
            ident_t = cpool.tile([P, P], F16, name="ident_t")
            ones_t = cpool.tile([P, 1], F16, name="ones_t")
            cos_t = cpool.tile([P, S], F16, name="cos_t")
            sin_t = cpool.tile([P, S], F16, name="sin_t")

            # ------------- persistent activation buffers -------------
            ppool = ctx.enter_context(tc.tile_pool(name="persist", bufs=1))
            qk = {}
            for n in range(QH + 1):
                for c in range(NS):
                    qk[(n, c)] = ppool.tile(
                        [P, SC], F16, name=f"qk{n}_{c}", tag=f"qk{n}_{c}"
                    )
            vT = [
                ppool.tile([P, SC], F16, name=f"vT{c}", tag=f"vT{c}")
                for c in range(NS)
            ]
            vnat = [
                ppool.tile([P, P], F16, name=f"vn{t}", tag=f"vn{t}")
                for t in range(ST)
            ]
            attnT = {}
            for h in range(QH):
                for c in range(NS):
                    attnT[(h, c)] = ppool.tile(
                        [P, SC], F16, name=f"at{h}_{c}", tag=f"at{h}_{c}"
                    )

            # ------------- resident weights (wqkv + wo, fp16) -------------
            # wqkv groups load first (phase A needs them immediately); the
            # small constants follow; wo loads are deferred until after
            # phase A emission so they don't delay the QKV pipeline.
            wpool = ctx.enter_context(tc.tile_pool(name="wres", bufs=1))
            wo_t = [
                wpool.tile([P, HID], F16, name=f"wo{hh}", tag=f"wo{hh}")
                for hh in range(QH)
            ]
            wq_g = {}

            def load_group(g):
                wq_g[g] = wpool.tile(
                    [P, GK * WCOLS], F16, name=f"wqg{g}", tag=f"wqg{g}"
                )
                src = wqkv[g * GK * P : (g + 1) * GK * P, :].rearrange(
                    "(t p) n -> p t n", p=P
                )
                dst = wq_g[g][:].rearrange("p (t n) -> p t n", t=GK)
                nc.sync.dma_start(dst, src)

            def wslice(k, n):
                g, kk = divmod(k, GK)
                off = kk * WCOLS + n * P
                return wq_g[g][:, off : off + P]

            for g in range(NG):
                load_group(g)
            nc.gpsimd.collective_compute(
                "ReduceScatter",
                mybir.AluOpType.add,
                replica_groups=groups,
                ins=[dum_in[:, :]],
                outs=[dum_out[:, :]],
            )
            nc.sync.dma_start(mask_t[:], maskd)
            nc.sync.dma_start(ident_t[:], ident)
            nc.sync.dma_start(ones_t[:], onesd)
            nc.sync.dma_start(cos_t[:], cos2)
            nc.sync.dma_start(sin_t[:], sinn2)

            # ---------------- stage A: QKV projection + RoPE + vT ------------
            half = P // 2
            with tc.tile_pool(name="ht", bufs=4) as h_pool, tc.tile_pool(
                name="psA", bufs=NQK, space="PSUM"
            ) as psA, tc.tile_pool(
                name="pst", bufs=2, space="PSUM"
            ) as pst, tc.tile_pool(name="ropet", bufs=3) as rpool:
                for c in range(NS):
                    ps = [
                        psA.tile([P, SC], F32, name=f"psA{n}", tag="psA")
                        for n in range(NQK)
                    ]
                    for k in range(KT):
                        ht_t = h_pool.tile([P, SC], F16, name="ht_t", tag="ht")
                        nc.sync.dma_start(
                            ht_t[:], hT[k * P : (k + 1) * P, c * SC : (c + 1) * SC]
                        )
                        for n in range(NQK):
                            nc.tensor.matmul(
                                ps[n][:],
                                wslice(k, n),
                                ht_t[:],
                                start=(k == 0),
                                stop=(k == KT - 1),
                            )
                    for n in range(QH + 1):
                        nc.scalar.copy(qk[(n, c)][:], ps[n][:])
                    nc.scalar.copy(vT[c][:], ps[NQK - 1][:])

                    # RoPE in place on this chunk's q heads and k (DVE),
                    # overlapped with the next chunk's QKV matmuls
                    csl = cos_t[:, c * SC : (c + 1) * SC]
                    ssl = sin_t[:, c * SC : (c + 1) * SC]
                    for n in range(QH + 1):
                        src = qk[(n, c)]
                        swp = rpool.tile([P, SC], F16, name="swp", tag="swp")
                        t1 = rpool.tile([P, SC], F16, name="t1", tag="t1")
                        nc.sync.dma_start(swp[0:half, :], src[half:P, :])
                        nc.sync.dma_start(swp[half:P, :], src[0:half, :])
                        nc.vector.tensor_mul(t1[:], src[:], csl)
                        nc.vector.tensor_mul(swp[:], swp[:], ssl)
                        nc.vector.tensor_add(src[:], t1[:], swp[:])

                # v transpose to natural [s, d]
                for t in range(ST):
                    c, j = divmod(t, NS)
                    tp = pst.tile([P, P], F16, name="tp", tag="tp")
                    nc.tensor.transpose(
                        tp[:], vT[c][:, j * P : (j + 1) * P], ident_t[:]
                    )
                    nc.scalar.copy(vnat[t][:], tp[:])

            # wo loads queue behind phase A's DMA stream (needed ~200us in)
            for hh in range(QH):
                nc.sync.dma_start(wo_t[hh][:], wo[hh * P : (hh + 1) * P, :])

            # ---------- attention + o_proj + reduce-scatter ----------
            with tc.tile_pool(name="pssc", bufs=2, space="PSUM") as ps_sc, tc.tile_pool(
                name="pssm", bufs=2, space="PSUM"
            ) as ps_sm, tc.tile_pool(
                name="pspv", bufs=2, space="PSUM"
            ) as ps_pv, tc.tile_pool(
                name="psop", bufs=2, space="PSUM"
            ) as ps_op, tc.tile_pool(name="expp", bufs=6) as ep, tc.tile_pool(
                name="esump", bufs=2
            ) as esp, tc.tile_pool(name="smallp", bufs=2) as sp, tc.tile_pool(
                name="stagep", bufs=32
            ) as stp:
                # stagep is deep on purpose: while a ReduceScatter is in
                # flight the SDMA engines starve regular DMA queues, so a
                # full chunk of partial-write DMAs (32 tiles) must be able
                # to back up without blocking the eviction engines.

                def emit_oproj_group(c, jj, nn, eng):
                    op = ps_op.tile([P, SC], F32, name="op", tag="op")
                    for h in range(QH):
                        nc.tensor.matmul(
                            op[:],
                            attnT[(h, c)][:, jj * P : (jj + 1) * P],
                            wo_t[h][:, nn * SC : (nn + 1) * SC],
                            start=(h == 0),
                            stop=(h == QH - 1),
                        )
                    st = stp.tile([P, SC], F16, name="st", tag="st")
                    # alternate eviction between Act and DVE to balance load
                    if eng == 0:
                        nc.scalar.copy(st[:], op[:])
                    else:
                        nc.vector.tensor_copy(st[:], op[:])
                    if c < NS - 1:
                        dst = partials[c]
                        row = jj * P
                    else:
                        dst = partials[NS - 1 + jj // 2]
                        row = (jj % 2) * P
                    nc.sync.dma_start(
                        dst[row : row + P, nn * SC : (nn + 1) * SC], st[:]
                    )

                def oproj_list(c):
                    return [(c, jj, nn) for jj in range(QH) for nn in range(HID // SC)]

                def emit_rs(idx):
                    # idx 0..2: full chunks; idx 3,4: halves of chunk 3
                    rows = 64 if idx < NS - 1 else 32
                    off = idx * 64 if idx < NS - 1 else 192 + (idx - NS + 1) * 32
                    nc.gpsimd.collective_compute(
                        "ReduceScatter",
                        mybir.AluOpType.add,
                        replica_groups=groups,
                        ins=[partials[idx][:, :]],
                        outs=[rs_outs[idx][:, :]],
                    )
                    nc.sync.dma_start(out[off : off + rows, :], rs_outs[idx][:, :])

                for c in range(NS):
                    prev = oproj_list(c - 1) if c > 0 else []
                    nsk = QH * c + QH  # causal: sk tiles for this chunk
                    total_steps = QH * nsk
                    oi = 0
                    si = 0
                    for h in range(QH):
                        esum = esp.tile([P, SC], F16, name="esum", tag="esum")
                        pv = ps_pv.tile([P, SC], F32, name="pv", tag="pv")
                        qrhs = qk[(h, c)][:]
                        for t in range(nsk):
                            kc, kj = divmod(t, NS)
                            ktile = qk[(QH, kc)][:, kj * P : (kj + 1) * P]
                            # diagonal tiles: columns below j*P are fully
                            # masked; compute only the live region
                            lo = (t - QH * c) * P if t >= QH * c else 0
                            sc_ps = ps_sc.tile([P, SC], F32, name="sc_ps", tag="sc")
                            nc.tensor.matmul(
                                sc_ps[:, lo:SC], ktile, qrhs[:, lo:SC],
                                start=True, stop=True,
                            )
                            e = ep.tile([P, SC], F16, name="e", tag="e")
                            nc.scalar.activation(
                                e[:, lo:SC], sc_ps[:, lo:SC], AF.Exp, scale=ISQRT_D
                            )
                            if t >= QH * c:
                                # triangular mask on the diagonal P-block
                                nc.vector.tensor_mul(
                                    e[:, lo : lo + P],
                                    e[:, lo : lo + P],
                                    mask_t[:, 3 * P : 4 * P],
                                )
                            if t == 0:
                                nc.vector.tensor_copy(esum[:], e[:])
                            else:
                                nc.vector.tensor_add(
                                    esum[:, lo:SC], esum[:, lo:SC], e[:, lo:SC]
                                )
                            nc.tensor.matmul(
                                pv[:, lo:SC], vnat[t][:], e[:, lo:SC],
                                start=(t == 0), stop=(t == nsk - 1),
                            )
                            si += 1
                            # interleave the previous chunk's o_proj: start
                            # a few steps in (its attnT normalization chain
                            # is still completing), finish by 3/4 of the
                            # loop so its ReduceScatter fires early
                            start_si = 6
                            end_si = max(start_si + 1, total_steps // 3)
                            frac = (si - start_si) / (end_si - start_si)
                            want = int(len(prev) * min(max(frac, 0.0), 1.0))
                            while prev and oi < want:
                                cc, jj, nn = prev[oi]
                                emit_oproj_group(cc, jj, nn, oi % 2)
                                oi += 1
                        sm = ps_sm.tile([1, SC], F32, name="sm", tag="sm")
                        nc.tensor.matmul(
                            sm[:], ones_t[:], esum[:], start=True, stop=True
                        )
                        # iterative-divide reciprocal is ~6 cyc/elem; the
                        # approx variant (~51 ULP) is 5x faster and far
                        # inside the 2e-2 budget. Reciprocate the [1,512]
                        # row, then broadcast the result.
                        smh = sp.tile([1, SC], F32, name="smh", tag="smh")
                        rcp = sp.tile([1, SC], F32, name="rcp", tag="rcp")
                        bc = sp.tile([P, SC], F32, name="bc", tag="bc")
                        nc.scalar.copy(smh[:], sm[:])
                        nc.vector.reciprocal_approx_fast(rcp[:], smh[:])
                        nc.gpsimd.partition_broadcast(bc[:], rcp[:])
                        nc.vector.tensor_mul(attnT[(h, c)][:], pv[:], bc[:])
                    while oi < len(prev):
                        cc, jj, nn = prev[oi]
                        emit_oproj_group(cc, jj, nn, oi % 2)
                        oi += 1
                    if c > 0:
                        emit_rs(c - 1)

                last = oproj_list(NS - 1)
                for idx, (cc, jj, nn) in enumerate(last):
                    emit_oproj_group(cc, jj, nn, idx % 2)
                    if idx == len(last) // 2 - 1:
                        emit_rs(NS - 1)  # first half of chunk 3
                emit_rs(NS)  # second half of chunk 3

    nc.compile()
    return nc


def _get_nc():
    if "nc" not in _CACHE:
        _CACHE["nc"] = _build()
    return _CACHE["nc"]


def _host_inputs(positions, hidden_states, Wqkv, Wo):
    """Shard + relayout the full inputs for the 8 cores (fp16 device side)."""
    pos = np.asarray(positions).reshape(-1).astype(np.float64)  # [S]
    hs = np.asarray(hidden_states, dtype=np.float32).reshape(S, HID)
    Wqkv = np.asarray(Wqkv, dtype=np.float32)
    Wo = np.asarray(Wo, dtype=np.float32)

    hT = np.ascontiguousarray(hs.T).astype(np.float16)  # [HID, S]

    half = D // 2
    inv_freq = 1.0 / (THETA ** (np.arange(half, dtype=np.float64) / half))
    ang = pos[None, :] * inv_freq[:, None]  # [64, S]
    cos = np.cos(ang)
    sin = np.sin(ang)
    cos2 = np.ascontiguousarray(np.concatenate([cos, cos], axis=0)).astype(
        np.float16
    )
    sinn2 = np.ascontiguousarray(np.concatenate([-sin, sin], axis=0)).astype(
        np.float16
    )

    # causal mask, [sk, sq] orientation: [zeros(128x384) | upper-tri(128x128)].
    maskd = np.concatenate(
        [np.zeros((P, 3 * P), dtype=np.float16),
         np.triu(np.ones((P, P), dtype=np.float16))], axis=1)
    ident = np.eye(P, dtype=np.float16)
    onesd = np.ones((P, 1), dtype=np.float16)

    qb = Wqkv[:, : H * D]
    kb = Wqkv[:, H * D : H * D + KVH * D]
    vb = Wqkv[:, H * D + KVH * D :]

    in_maps = []
    for c in range(NCORES):
        wq_c = np.concatenate(
            [
                qb[:, c * QH * D : (c + 1) * QH * D],
                kb[:, c * D : (c + 1) * D],
                vb[:, c * D : (c + 1) * D],
            ],
            axis=1,
        ).astype(np.float16)
        wo_c = Wo[c * QH * D : (c + 1) * QH * D, :].astype(np.float16)
        in_maps.append(
            {
                "hT": hT,
                "wqkv": np.ascontiguousarray(wq_c),
                "wo": np.ascontiguousarray(wo_c),
                "cos2": cos2,
                "sinn2": sinn2,
                "maskd": maskd,
                "ident": ident,
                "onesd": onesd,
            }
        )
    return in_maps


def _assemble(results):
    full = np.empty((S, HID), dtype=np.float32)
    for c in range(NCORES):
        oc = np.asarray(results[c]["out"], dtype=np.float32)  # [256, HID]
        for j in range(NS - 1):
            full[SC * j + 64 * c : SC * j + 64 * (c + 1), :] = oc[
                64 * j : 64 * (j + 1), :
            ]
        # chunk 3 was reduce-scattered as two 256-row halves
        full[3 * SC + 32 * c : 3 * SC + 32 * (c + 1), :] = oc[192:224, :]
        full[3 * SC + 256 + 32 * c : 3 * SC + 256 + 32 * (c + 1), :] = oc[
            224:256, :
        ]
    return full.reshape(1, S, HID)


def kernel(positions, hidden_states, Wqkv, Wo):
    from concourse.bass_utils import run_bass_kernel_spmd

    nc = _get_nc()
    in_maps = _host_inputs(positions, hidden_states, Wqkv, Wo)
    res = run_bass_kernel_spmd(nc, in_maps, core_ids=list(range(NCORES)))
    return _assemble(res.results)


def kernel_timed(positions, hidden_states, Wqkv, Wo, tmpdir="/tmp/ntff_trace"):
    """Like kernel() but with NTFF profiling; returns (output, exec_time_ns)."""
    import os
    import shutil

    from concourse.bass_utils import run_bass_kernel_spmd

    shutil.rmtree(tmpdir, ignore_errors=True)
    os.makedirs(tmpdir, exist_ok=True)
    nc = _get_nc()
    in_maps = _host_inputs(positions, hidden_states, Wqkv, Wo)
    res = run_bass_kernel_spmd(
        nc, in_maps, core_ids=list(range(NCORES)), trace=True, tmpdir=tmpdir
    )
    return _assemble(res.results), res.exec_time_ns


# revision 30
# speedup vs baseline: 1.1941x; 1.0026x over previous
"""Llama attention layer (B=1, S=2048, H=32, KVH=8, D=128, HID=4096) on 8 TRN2
NeuronCores.

Sharding: tensor-parallel over head groups. Core c computes Q heads
[4c..4c+4) and KV head c end-to-end (QKV projection, RoPE, causal GQA
attention, o_proj rows for its heads), then a chunked ReduceScatter sums the
o_proj partials so core c ends up with rows {512j + 64c .. 512j + 64c + 64}
of the output for j in 0..3. The host reassembles the full [2048, 4096]
output by concatenating the shards.

v2 design (fp16 end-to-end, pipelined):
  - All DRAM-resident tensors are fp16 (half the HBM traffic and half the
    collective bytes of fp32; fp16 matmuls run at full PE rate and carry
    10-bit mantissas). PSUM accumulation stays fp32.
  - Single-pass QKV: the whole wqkv shard (6.3 MB fp16) is SBUF-resident,
    hT streams through once. RoPE runs on DVE per chunk, overlapped with
    the next chunk's QKV matmuls.
  - Softmax denominators accumulate on DVE (esum += exp tile) instead of
    per-tile PE ones-matmuls; one [1,512] ones-matmul per (chunk, head)
    reduces esum across partitions.
  - o_proj matmul groups of chunk c-1 are interleaved into the attention
    t-loop of chunk c so the PE never idles waiting on the scalar engine's
    exp tiles.
  - Per-chunk fp16 ReduceScatter overlaps the remaining compute.
"""

import sys

if "/opt/trn_rl_repo" not in sys.path:
    sys.path.insert(0, "/opt/trn_rl_repo")

import numpy as np

# Model dims (hardcoded per problem spec)
H, KVH, D, HID = 32, 8, 128, 4096
S = 2048
THETA = 10000.0
NCORES = 8
QH = H // NCORES          # 4 query heads per core
P = 128                   # partitions
SC = 512                  # sequence chunk (matmul free dim)
NS = S // SC              # 4 chunks
KT = HID // P             # 32 contraction tiles for the projections
ST = S // P               # 16 sequence tiles of 128
NQK = QH + 2              # col-tiles per core in wqkv: q0..q3, k, v
WCOLS = NQK * P           # 768
GK = 4                    # weight k-tiles per DMA group
NG = KT // GK             # 8 groups
ISQRT_D = float(D) ** -0.5

_CACHE = {}


def _build():
    import concourse.bass as bass
    import concourse.tile as tile
    from concourse import bacc, mybir
    from contextlib import ExitStack

    F32 = mybir.dt.float32
    F32R = mybir.dt.float32r
    F16 = mybir.dt.float16
    AF = mybir.ActivationFunctionType

    nc = bacc.Bacc(
        "TRN2",
        target_bir_lowering=False,
        debug=False,
        enable_asserts=True,
        num_devices=NCORES,
    )

    hT = nc.dram_tensor("hT", [HID, S], F16, kind="ExternalInput").ap()
    wqkv = nc.dram_tensor("wqkv", [HID, WCOLS], F16, kind="ExternalInput").ap()
    wo = nc.dram_tensor("wo", [QH * D, HID], F16, kind="ExternalInput").ap()
    cos2 = nc.dram_tensor("cos2", [P, S], F16, kind="ExternalInput").ap()
    sinn2 = nc.dram_tensor("sinn2", [P, S], F16, kind="ExternalInput").ap()
    maskd = nc.dram_tensor("maskd", [P, 4 * P], F16, kind="ExternalInput").ap()
    ident = nc.dram_tensor("ident", [P, P], F16, kind="ExternalInput").ap()
    onesd = nc.dram_tensor("onesd", [P, 1], F16, kind="ExternalInput").ap()
    out = nc.dram_tensor("out", [S // NCORES, HID], F16, kind="ExternalOutput").ap()
    # per-chunk partial / rs tensors: separate DRAM tensors so the
    # whole-tensor WAR tracking never serializes chunk c+1's o_proj DMA
    # writes behind chunk c's in-flight ReduceScatter. The last chunk is
    # split in half to shorten the exposed collective tail.
    partials = [
        nc.dram_tensor(f"partial{c}", [SC, HID], F16).ap() for c in range(NS - 1)
    ]
    partials += [
        nc.dram_tensor("partial3a", [SC // 2, HID], F16).ap(),
        nc.dram_tensor("partial3b", [SC // 2, HID], F16).ap(),
    ]
    rs_outs = [
        nc.dram_tensor(f"rs{c}", [64, HID], F16).ap() for c in range(NS - 1)
    ]
    rs_outs += [
        nc.dram_tensor("rs3a", [32, HID], F16).ap(),
        nc.dram_tensor("rs3b", [32, HID], F16).ap(),
    ]
    # tiny dummy collective: absorbs the first-call warmup (~15us slower
    # algbw) during phase A instead of on the critical RS0
    dum_in = nc.dram_tensor("dum_in", [NCORES, 512], F16).ap()
    dum_out = nc.dram_tensor("dum_out", [1, 512], F16).ap()

    groups = [list(range(NCORES))]

    with tile.TileContext(nc) as tc:
        with ExitStack() as ctx:
            # ---------------- constants (whole-kernel lifetime) ----------------
            cpool = ctx.enter_context(tc.tile_pool(name="const", bufs=1))
            mask_t = cpool.tile([P, 4 * P], F16, name="mask_t")
            ident_t = cpool.tile([P, P], F16, name="ident_t")
            ones_t = cpool.tile([P, 1], F16, name="ones_t")
            cos_t = cpool.tile([P, S], F16, name="cos_t")
            sin_t = cpool.tile([P, S], F16, name="sin_t")

            # ------------- persistent activation buffers -------------
            ppool = ctx.enter_context(tc.tile_pool(name="persist", bufs=1))
            qk = {}
            for n in range(QH + 1):
                for c in range(NS):
                    qk[(n, c)] = ppool.tile(
                        [P, SC], F16, name=f"qk{n}_{c}", tag=f"qk{n}_{c}"
                    )
            vT = [
                ppool.tile([P, SC], F16, name=f"vT{c}", tag=f"vT{c}")
                for c in range(NS)
            ]
            vnat = [
                ppool.tile([P, P], F16, name=f"vn{t}", tag=f"vn{t}")
                for t in range(ST)
            ]
            attnT = {}
            for h in range(QH):
                for c in range(NS):
                    attnT[(h, c)] = ppool.tile(
                        [P, SC], F16, name=f"at{h}_{c}", tag=f"at{h}_{c}"
                    )

            # ------------- resident weights (wqkv + wo, fp16) -------------
            # wqkv groups load first (phase A needs them immediately); the
            # small constants follow; wo loads are deferred until after
            # phase A emission so they don't delay the QKV pipeline.
            wpool = ctx.enter_context(tc.tile_pool(name="wres", bufs=1))
            wo_t = [
                wpool.tile([P, HID], F16, name=f"wo{hh}", tag=f"wo{hh}")
                for hh in range(QH)
            ]
            wq_g = {}

            def load_group(g):
                wq_g[g] = wpool.tile(
                    [P, GK * WCOLS], F16, name=f"wqg{g}", tag=f"wqg{g}"
                )
                src = wqkv[g * GK * P : (g + 1) * GK * P, :].rearrange(
                    "(t p) n -> p t n", p=P
                )
                dst = wq_g[g][:].rearrange("p (t n) -> p t n", t=GK)
                nc.sync.dma_start(dst, src)

            def wslice(k, n):
                g, kk = divmod(k, GK)
                off = kk * WCOLS + n * P
                return wq_g[g][:, off : off + P]

            for g in range(NG):
                load_group(g)
            nc.gpsimd.collective_compute(
                "ReduceScatter",
                mybir.AluOpType.add,
                replica_groups=groups,
                ins=[dum_in[:, :]],
                outs=[dum_out[:, :]],
            )
            nc.sync.dma_start(mask_t[:], maskd)
            nc.sync.dma_start(ident_t[:], ident)
            nc.sync.dma_start(ones_t[:], onesd)
            nc.sync.dma_start(cos_t[:], cos2)
            nc.sync.dma_start(sin_t[:], sinn2)

            # ---------------- stage A: QKV projection + RoPE + vT ------------
            half = P // 2
            with tc.tile_pool(name="ht", bufs=4) as h_pool, tc.tile_pool(
                name="psA", bufs=NQK, space="PSUM"
            ) as psA, tc.tile_pool(
                name="pst", bufs=2, space="PSUM"
            ) as pst, tc.tile_pool(name="ropet", bufs=3) as rpool:
                for c in range(NS):
                    ps = [
                        psA.tile([P, SC], F32, name=f"psA{n}", tag="psA")
                        for n in range(NQK)
                    ]
                    for k in range(KT):
                        ht_t = h_pool.tile([P, SC], F16, name="ht_t", tag="ht")
                        nc.sync.dma_start(
                            ht_t[:], hT[k * P : (k + 1) * P, c * SC : (c + 1) * SC]
                        )
                        for n in range(NQK):
                            nc.tensor.matmul(
                                ps[n][:],
                                wslice(k, n),
                                ht_t[:],
                                start=(k == 0),
                                stop=(k == KT - 1),
                            )
                    for n in range(QH + 1):
                        nc.scalar.copy(qk[(n, c)][:], ps[n][:])
                    nc.scalar.copy(vT[c][:], ps[NQK - 1][:])

                    # RoPE in place on this chunk's q heads and k (DVE),
                    # overlapped with the next chunk's QKV matmuls
                    csl = cos_t[:, c * SC : (c + 1) * SC]
                    ssl = sin_t[:, c * SC : (c + 1) * SC]
                    for n in range(QH + 1):
                        src = qk[(n, c)]
                        swp = rpool.tile([P, SC], F16, name="swp", tag="swp")
                        t1 = rpool.tile([P, SC], F16, name="t1", tag="t1")
                        nc.sync.dma_start(swp[0:half, :], src[half:P, :])
                        nc.sync.dma_start(swp[half:P, :], src[0:half, :])
                        nc.vector.tensor_mul(t1[:], src[:], csl)
                        nc.vector.tensor_mul(swp[:], swp[:], ssl)
                        nc.vector.tensor_add(src[:], t1[:], swp[:])

                # v transpose to natural [s, d]
                for t in range(ST):
                    c, j = divmod(t, NS)
                    tp = pst.tile([P, P], F16, name="tp", tag="tp")
                    nc.tensor.transpose(
                        tp[:], vT[c][:, j * P : (j + 1) * P], ident_t[:]
                    )
                    nc.scalar.copy(vnat[t][:], tp[:])

            # wo loads queue behind phase A's DMA stream (needed ~200us in)
            for hh in range(QH):
                nc.sync.dma_start(wo_t[hh][:], wo[hh * P : (hh + 1) * P, :])

            # ---------- attention + o_proj + reduce-scatter ----------
            with tc.tile_pool(name="pssc", bufs=2, space="PSUM") as ps_sc, tc.tile_pool(
                name="pssm", bufs=2, space="PSUM"
            ) as ps_sm, tc.tile_pool(
                name="pspv", bufs=2, space="PSUM"
            ) as ps_pv, tc.tile_pool(
                name="psop", bufs=2, space="PSUM"
            ) as ps_op, tc.tile_pool(name="expp", bufs=6) as ep, tc.tile_pool(
                name="esump", bufs=2
            ) as esp, tc.tile_pool(name="smallp", bufs=2) as sp, tc.tile_pool(
                name="stagep", bufs=32
            ) as stp:
                # stagep is deep on purpose: while a ReduceScatter is in
                # flight the SDMA engines starve regular DMA queues, so a
                # full chunk of partial-write DMAs (32 tiles) must be able
                # to back up without blocking the eviction engines.

                def emit_oproj_group(c, jj, nn, eng):
                    op = ps_op.tile([P, SC], F32, name="op", tag="op")
                    for h in range(QH):
                        nc.tensor.matmul(
                            op[:],
                            attnT[(h, c)][:, jj * P : (jj + 1) * P],
                            wo_t[h][:, nn * SC : (nn + 1) * SC],
                            start=(h == 0),
                            stop=(h == QH - 1),
                        )
                    st = stp.tile([P, SC], F16, name="st", tag="st")
                    # alternate eviction between Act and DVE to balance load
                    if eng == 0:
                        nc.scalar.copy(st[:], op[:])
                    else:
                        nc.vector.tensor_copy(st[:], op[:])
                    if c < NS - 1:
                        dst = partials[c]
                        row = jj * P
                    else:
                        dst = partials[NS - 1 + jj // 2]
                        row = (jj % 2) * P
                    nc.sync.dma_start(
                        dst[row : row + P, nn * SC : (nn + 1) * SC], st[:]
                    )

                def oproj_list(c):
                    return [(c, jj, nn) for jj in range(QH) for nn in range(HID // SC)]

                def emit_rs(idx):
                    # idx 0..2: full chunks; idx 3,4: halves of chunk 3
                    rows = 64 if idx < NS - 1 else 32
                    off = idx * 64 if idx < NS - 1 else 192 + (idx - NS + 1) * 32
                    nc.gpsimd.collective_compute(
                        "ReduceScatter",
                        mybir.AluOpType.add,
                        replica_groups=groups,
                        ins=[partials[idx][:, :]],
                        outs=[rs_outs[idx][:, :]],
                    )
                    nc.sync.dma_start(out[off : off + rows, :], rs_outs[idx][:, :])

                for c in range(NS):
                    prev = oproj_list(c - 1) if c > 0 else []
                    nsk = QH * c + QH  # causal: sk tiles for this chunk
                    total_steps = QH * nsk
                    oi = 0
                    si = 0
                    for h in range(QH):
                        esum = esp.tile([P, SC], F16, name="esum", tag="esum")
                        pv = ps_pv.tile([P, SC], F32, name="pv", tag="pv")
                        qrhs = qk[(h, c)][:]
                        for t in range(nsk):
                            kc, kj = divmod(t, NS)
                            ktile = qk[(QH, kc)][:, kj * P : (kj + 1) * P]
                            # diagonal tiles: columns below j*P are fully
                            # masked; compute only the live region
                            lo = (t - QH * c) * P if t >= QH * c else 0
                            sc_ps = ps_sc.tile([P, SC], F32, name="sc_ps", tag="sc")
                            nc.tensor.matmul(
                                sc_ps[:, lo:SC], ktile, qrhs[:, lo:SC],
                                start=True, stop=True,
                            )
                            e = ep.tile([P, SC], F16, name="e", tag="e")
                            nc.scalar.activation(
                                e[:, lo:SC], sc_ps[:, lo:SC], AF.Exp, scale=ISQRT_D
                            )
                            if t >= QH * c:
                                # triangular mask on the diagonal P-block
                                nc.vector.tensor_mul(
                                    e[:, lo : lo + P],
                                    e[:, lo : lo + P],
                                    mask_t[:, 3 * P : 4 * P],
                                )
                            if t == 0:
                                nc.vector.tensor_copy(esum[:], e[:])
                            else:
                                nc.vector.tensor_add(
                                    esum[:, lo:SC], esum[:, lo:SC], e[:, lo:SC]
                                )
                            nc.tensor.matmul(
                                pv[:, lo:SC], vnat[t][:], e[:, lo:SC],
                                start=(t == 0), stop=(t == nsk - 1),
                            )
                            si += 1
                            # interleave the previous chunk's o_proj: start
                            # a few steps in (its attnT normalization chain
                            # is still completing), finish early so its
                            # ReduceScatter fires early
                            start_si = 6
                            end_si = max(start_si + 1, total_steps // 3)
                            frac = (si - start_si) / (end_si - start_si)
                            want = int(len(prev) * min(max(frac, 0.0), 1.0))
                            while prev and oi < want:
                                cc, jj, nn = prev[oi]
                                emit_oproj_group(cc, jj, nn, oi % 2)
                                oi += 1
                        sm = ps_sm.tile([1, SC], F32, name="sm", tag="sm")
                        nc.tensor.matmul(
                            sm[:], ones_t[:], esum[:], start=True, stop=True
                        )
                        # iterative-divide reciprocal is ~6 cyc/elem; the
                        # approx variant (~51 ULP) is 5x faster and far
                        # inside the 2e-2 budget. Reciprocate the [1,512]
                        # row, then broadcast the result.
                        smh = sp.tile([1, SC], F32, name="smh", tag="smh")
                        rcp = sp.tile([1, SC], F32, name="rcp", tag="rcp")
                        bc = sp.tile([P, SC], F32, name="bc", tag="bc")
                        nc.scalar.copy(smh[:], sm[:])
                        nc.vector.reciprocal_approx_fast(rcp[:], smh[:])
                        nc.gpsimd.partition_broadcast(bc[:], rcp[:])
                        nc.vector.tensor_mul(attnT[(h, c)][:], pv[:], bc[:])
                    while oi < len(prev):
                        cc, jj, nn = prev[oi]
                        emit_oproj_group(cc, jj, nn, oi % 2)
                        oi += 1
                    if c > 0:
                        emit_rs(c - 1)

                last = oproj_list(NS - 1)
                for idx, (cc, jj, nn) in enumerate(last):
                    emit_oproj_group(cc, jj, nn, idx % 2)
                    if idx == len(last) // 2 - 1:
                        emit_rs(NS - 1)  # first half of chunk 3
                emit_rs(NS)  # second half of chunk 3

    nc.compile()
    return nc


def _get_nc():
    if "nc" not in _CACHE:
        _CACHE["nc"] = _build()
    return _CACHE["nc"]


def _host_inputs(positions, hidden_states, Wqkv, Wo):
    """Shard + relayout the full inputs for the 8 cores (fp16 device side)."""
    pos = np.asarray(positions).reshape(-1).astype(np.float64)  # [S]
    hs = np.asarray(hidden_states, dtype=np.float32).reshape(S, HID)
    Wqkv = np.asarray(Wqkv, dtype=np.float32)
    Wo = np.asarray(Wo, dtype=np.float32)

    hT = np.ascontiguousarray(hs.T).astype(np.float16)  # [HID, S]

    half = D // 2
    inv_freq = 1.0 / (THETA ** (np.arange(half, dtype=np.float64) / half))
    ang = pos[None, :] * inv_freq[:, None]  # [64, S]
    cos = np.cos(ang)
    sin = np.sin(ang)
    cos2 = np.ascontiguousarray(np.concatenate([cos, cos], axis=0)).astype(
        np.float16
    )
    sinn2 = np.ascontiguousarray(np.concatenate([-sin, sin], axis=0)).astype(
        np.float16
    )

    # causal mask, [sk, sq] orientation: [zeros(128x384) | upper-tri(128x128)].
    maskd = np.concatenate(
        [np.zeros((P, 3 * P), dtype=np.float16),
         np.triu(np.ones((P, P), dtype=np.float16))], axis=1)
    ident = np.eye(P, dtype=np.float16)
    onesd = np.ones((P, 1), dtype=np.float16)

    qb = Wqkv[:, : H * D]
    kb = Wqkv[:, H * D : H * D + KVH * D]
    vb = Wqkv[:, H * D + KVH * D :]

    in_maps = []
    for c in range(NCORES):
        wq_c = np.concatenate(
            [
                qb[:, c * QH * D : (c + 1) * QH * D],
                kb[:, c * D : (c + 1) * D],
                vb[:, c * D : (c + 1) * D],
            ],
            axis=1,
        ).astype(np.float16)
        wo_c = Wo[c * QH * D : (c + 1) * QH * D, :].astype(np.float16)
        in_maps.append(
            {
                "hT": hT,
                "wqkv": np.ascontiguousarray(wq_c),
                "wo": np.ascontiguousarray(wo_c),
                "cos2": cos2,
                "sinn2": sinn2,
                "maskd": maskd,
                "ident": ident,
                "onesd": onesd,
            }
        )
    return in_maps


def _assemble(results):
    full = np.empty((S, HID), dtype=np.float32)
    for c in range(NCORES):
        oc = np.asarray(results[c]["out"], dtype=np.float32)  # [256, HID]
        for j in range(NS - 1):
            full[SC * j + 64 * c : SC * j + 64 * (c + 1), :] = oc[
                64 * j : 64 * (j + 1), :
            ]
        # chunk 3 was reduce-scattered as two 256-row halves
        full[3 * SC + 32 * c : 3 * SC + 32 * (c + 1), :] = oc[192:224, :]
        full[3 * SC + 256 + 32 * c : 3 * SC + 256 + 32 * (c + 1), :] = oc[
            224:256, :
        ]
    return full.reshape(1, S, HID)


def kernel(positions, hidden_states, Wqkv, Wo):
    from concourse.bass_utils import run_bass_kernel_spmd

    nc = _get_nc()
    in_maps = _host_inputs(positions, hidden_states, Wqkv, Wo)
    res = run_bass_kernel_spmd(nc, in_maps, core_ids=list(range(NCORES)))
    return _assemble(res.results)


def kernel_timed(positions, hidden_states, Wqkv, Wo, tmpdir="/tmp/ntff_trace"):
    """Like kernel() but with NTFF profiling; returns (output, exec_time_ns)."""
    import os
    import shutil

    from concourse.bass_utils import run_bass_kernel_spmd

    shutil.rmtree(tmpdir, ignore_errors=True)
    os.makedirs(tmpdir, exist_ok=True)
    nc = _get_nc()
    in_maps = _host_inputs(positions, hidden_states, Wqkv, Wo)
    res = run_bass_kernel_spmd(
        nc, in_maps, core_ids=list(range(NCORES)), trace=True, tmpdir=tmpdir
    )
    return _assemble(res.results), res.exec_time_ns


# revision 37
# speedup vs baseline: 1.2022x; 1.0068x over previous
"""Llama attention layer (B=1, S=2048, H=32, KVH=8, D=128, HID=4096) on 8 TRN2
NeuronCores.

Sharding: tensor-parallel over head groups. Core c computes Q heads
[4c..4c+4) and KV head c end-to-end (QKV projection, RoPE, causal GQA
attention, o_proj rows for its heads), then a chunked ReduceScatter sums the
o_proj partials so core c ends up with rows {512j + 64c .. 512j + 64c + 64}
of the output for j in 0..3. The host reassembles the full [2048, 4096]
output by concatenating the shards.

v2 design (fp16 end-to-end, pipelined):
  - All DRAM-resident tensors are fp16 (half the HBM traffic and half the
    collective bytes of fp32; fp16 matmuls run at full PE rate and carry
    10-bit mantissas). PSUM accumulation stays fp32.
  - Single-pass QKV: the whole wqkv shard (6.3 MB fp16) is SBUF-resident,
    hT streams through once. RoPE runs on DVE per chunk, overlapped with
    the next chunk's QKV matmuls.
  - Softmax denominators accumulate on DVE (esum += exp tile) instead of
    per-tile PE ones-matmuls; one [1,512] ones-matmul per (chunk, head)
    reduces esum across partitions.
  - o_proj matmul groups of chunk c-1 are interleaved into the attention
    t-loop of chunk c so the PE never idles waiting on the scalar engine's
    exp tiles.
  - Per-chunk fp16 ReduceScatter overlaps the remaining compute.
"""

import sys

if "/opt/trn_rl_repo" not in sys.path:
    sys.path.insert(0, "/opt/trn_rl_repo")

import numpy as np

# Model dims (hardcoded per problem spec)
H, KVH, D, HID = 32, 8, 128, 4096
S = 2048
THETA = 10000.0
NCORES = 8
QH = H // NCORES          # 4 query heads per core
P = 128                   # partitions
SC = 512                  # sequence chunk (matmul free dim)
NS = S // SC              # 4 chunks
KT = HID // P             # 32 contraction tiles for the projections
ST = S // P               # 16 sequence tiles of 128
NQK = QH + 2              # col-tiles per core in wqkv: q0..q3, k, v
WCOLS = NQK * P           # 768
GK = 2                    # weight k-tiles per DMA group
NG = KT // GK             # 16 groups
ISQRT_D = float(D) ** -0.5

_CACHE = {}


def _build():
    import concourse.bass as bass
    import concourse.tile as tile
    from concourse import bacc, mybir
    from contextlib import ExitStack

    F32 = mybir.dt.float32
    F32R = mybir.dt.float32r
    F16 = mybir.dt.float16
    AF = mybir.ActivationFunctionType

    nc = bacc.Bacc(
        "TRN2",
        target_bir_lowering=False,
        debug=False,
        enable_asserts=False,
        num_devices=NCORES,
    )

    hT = nc.dram_tensor("hT", [HID, S], F16, kind="ExternalInput").ap()
    wqkv = nc.dram_tensor("wqkv", [HID, WCOLS], F16, kind="ExternalInput").ap()
    wo = nc.dram_tensor("wo", [QH * D, HID], F16, kind="ExternalInput").ap()
    cos2 = nc.dram_tensor("cos2", [P, S], F16, kind="ExternalInput").ap()
    sinn2 = nc.dram_tensor("sinn2", [P, S], F16, kind="ExternalInput").ap()
    maskd = nc.dram_tensor("maskd", [P, 4 * P], F16, kind="ExternalInput").ap()
    ident = nc.dram_tensor("ident", [P, P], F16, kind="ExternalInput").ap()
    onesd = nc.dram_tensor("onesd", [P, 1], F16, kind="ExternalInput").ap()
    out = nc.dram_tensor("out", [S // NCORES, HID], F16, kind="ExternalOutput").ap()
    # per-chunk partial / rs tensors: separate DRAM tensors so the
    # whole-tensor WAR tracking never serializes chunk c+1's o_proj DMA
    # writes behind chunk c's in-flight ReduceScatter.
    partials = [
        nc.dram_tensor(f"partial{c}", [SC, HID], F16).ap() for c in range(NS)
    ]
    rs_outs = [
        nc.dram_tensor(f"rs{c}", [64, HID], F16).ap() for c in range(NS)
    ]
    # dummy collective: absorbs the first-call warmup (slow algbw ramp)
    # during phase A instead of on the critical RS0
    dum_in = nc.dram_tensor("dum_in", [SC, SC], F16).ap()
    dum_out = nc.dram_tensor("dum_out", [64, SC], F16).ap()

    groups = [list(range(NCORES))]

    with tile.TileContext(nc) as tc:
        with ExitStack() as ctx:
            # ---------------- constants (whole-kernel lifetime) ----------------
            cpool = ctx.enter_context(tc.tile_pool(name="const", bufs=1))
            mask_t = cpool.tile([P, 4 * P], F16, name="mask_t")
            ident_t = cpool.tile([P, P], F16, name="ident_t")
            ones_t = cpool.tile([P, 1], F16, name="ones_t")
            cos_t = cpool.tile([P, S], F16, name="cos_t")
            sin_t = cpool.tile([P, S], F16, name="sin_t")

            # ------------- persistent activation buffers -------------
            ppool = ctx.enter_context(tc.tile_pool(name="persist", bufs=1))
            qk = {}
            for n in range(QH + 1):
                for c in range(NS):
                    qk[(n, c)] = ppool.tile(
                        [P, SC], F16, name=f"qk{n}_{c}", tag=f"qk{n}_{c}"
                    )
            vT = [
                ppool.tile([P, SC], F16, name=f"vT{c}", tag=f"vT{c}")
                for c in range(NS)
            ]
            vnat = [
                ppool.tile([P, P], F16, name=f"vn{t}", tag=f"vn{t}")
                for t in range(ST)
            ]
            attnT = {}
            for h in range(QH):
                for c in range(NS):
                    attnT[(h, c)] = ppool.tile(
                        [P, SC], F16, name=f"at{h}_{c}", tag=f"at{h}_{c}"
                    )

            # ------------- resident weights (wqkv + wo, fp16) -------------
            # wqkv groups load first (phase A needs them immediately); the
            # small constants follow; wo loads are deferred until after
            # phase A emission so they don't delay the QKV pipeline.
            wpool = ctx.enter_context(tc.tile_pool(name="wres", bufs=1))
            wo_t = [
                wpool.tile([P, HID], F16, name=f"wo{hh}", tag=f"wo{hh}")
                for hh in range(QH)
            ]
            wq_g = {}

            def load_group(g):
                wq_g[g] = wpool.tile(
                    [P, GK * WCOLS], F16, name=f"wqg{g}", tag=f"wqg{g}"
                )
                src = wqkv[g * GK * P : (g + 1) * GK * P, :].rearrange(
                    "(t p) n -> p t n", p=P
                )
                dst = wq_g[g][:].rearrange("p (t n) -> p t n", t=GK)
                nc.sync.dma_start(dst, src)

            def wslice(k, n):
                g, kk = divmod(k, GK)
                off = kk * WCOLS + n * P
                return wq_g[g][:, off : off + P]

            for g in range(NG):
                load_group(g)
            nc.gpsimd.collective_compute(
                "ReduceScatter",
                mybir.AluOpType.add,
                replica_groups=groups,
                ins=[dum_in[:, :]],
                outs=[dum_out[:, :]],
            )
            nc.sync.dma_start(mask_t[:], maskd)
            nc.sync.dma_start(ident_t[:], ident)
            nc.sync.dma_start(ones_t[:], onesd)
            nc.sync.dma_start(cos_t[:], cos2)
            nc.sync.dma_start(sin_t[:], sinn2)

            # ---------------- stage A: QKV projection + RoPE + vT ------------
            half = P // 2
            with tc.tile_pool(name="ht", bufs=4) as h_pool, tc.tile_pool(
                name="psA", bufs=NQK, space="PSUM"
            ) as psA, tc.tile_pool(
                name="pst", bufs=2, space="PSUM"
            ) as pst, tc.tile_pool(name="ropet", bufs=3) as rpool:
                for c in range(NS):
                    ps = [
                        psA.tile([P, SC], F32, name=f"psA{n}", tag="psA")
                        for n in range(NQK)
                    ]
                    for k in range(KT):
                        ht_t = h_pool.tile([P, SC], F16, name="ht_t", tag="ht")
                        nc.sync.dma_start(
                            ht_t[:], hT[k * P : (k + 1) * P, c * SC : (c + 1) * SC]
                        )
                        for n in range(NQK):
                            nc.tensor.matmul(
                                ps[n][:],
                                wslice(k, n),
                                ht_t[:],
                                start=(k == 0),
                                stop=(k == KT - 1),
                            )
                    for n in range(QH + 1):
                        nc.scalar.copy(qk[(n, c)][:], ps[n][:])
                    nc.scalar.copy(vT[c][:], ps[NQK - 1][:])

                    # RoPE in place on this chunk's q heads and k (DVE),
                    # overlapped with the next chunk's QKV matmuls
                    csl = cos_t[:, c * SC : (c + 1) * SC]
                    ssl = sin_t[:, c * SC : (c + 1) * SC]
                    for n in range(QH + 1):
                        src = qk[(n, c)]
                        swp = rpool.tile([P, SC], F16, name="swp", tag="swp")
                        t1 = rpool.tile([P, SC], F16, name="t1", tag="t1")
                        nc.sync.dma_start(swp[0:half, :], src[half:P, :])
                        nc.sync.dma_start(swp[half:P, :], src[0:half, :])
                        nc.vector.tensor_mul(t1[:], src[:], csl)
                        nc.vector.tensor_mul(swp[:], swp[:], ssl)
                        nc.vector.tensor_add(src[:], t1[:], swp[:])

                # v transpose to natural [s, d]
                for t in range(ST):
                    c, j = divmod(t, NS)
                    tp = pst.tile([P, P], F16, name="tp", tag="tp")
                    nc.tensor.transpose(
                        tp[:], vT[c][:, j * P : (j + 1) * P], ident_t[:]
                    )
                    nc.scalar.copy(vnat[t][:], tp[:])

            # wo loads queue behind phase A's DMA stream (needed ~200us in)
            for hh in range(QH):
                nc.sync.dma_start(wo_t[hh][:], wo[hh * P : (hh + 1) * P, :])

            # ---------- attention + o_proj + reduce-scatter ----------
            with tc.tile_pool(name="pssc", bufs=2, space="PSUM") as ps_sc, tc.tile_pool(
                name="pssm", bufs=2, space="PSUM"
            ) as ps_sm, tc.tile_pool(
                name="pspv", bufs=2, space="PSUM"
            ) as ps_pv, tc.tile_pool(
                name="psop", bufs=2, space="PSUM"
            ) as ps_op, tc.tile_pool(name="expp", bufs=6) as ep, tc.tile_pool(
                name="esump", bufs=2
            ) as esp, tc.tile_pool(name="smallp", bufs=2) as sp, tc.tile_pool(
                name="stagep", bufs=32
            ) as stp:
                # stagep is deep on purpose: while a ReduceScatter is in
                # flight the SDMA engines starve regular DMA queues, so a
                # full chunk of partial-write DMAs (32 tiles) must be able
                # to back up without blocking the eviction engines.

                def emit_oproj_group(c, jj, nn, eng):
                    op = ps_op.tile([P, SC], F32, name="op", tag="op")
                    for h in range(QH):
                        nc.tensor.matmul(
                            op[:],
                            attnT[(h, c)][:, jj * P : (jj + 1) * P],
                            wo_t[h][:, nn * SC : (nn + 1) * SC],
                            start=(h == 0),
                            stop=(h == QH - 1),
                        )
                    st = stp.tile([P, SC], F16, name="st", tag="st")
                    # alternate eviction between Act and DVE to balance load
                    if eng == 0:
                        nc.scalar.copy(st[:], op[:])
                    else:
                        nc.vector.tensor_copy(st[:], op[:])
                    nc.sync.dma_start(
                        partials[c][jj * P : (jj + 1) * P, nn * SC : (nn + 1) * SC],
                        st[:],
                    )

                def oproj_list(c):
                    return [(c, jj, nn) for jj in range(QH) for nn in range(HID // SC)]

                def emit_rs(idx):
                    nc.gpsimd.collective_compute(
                        "ReduceScatter",
                        mybir.AluOpType.add,
                        replica_groups=groups,
                        ins=[partials[idx][:, :]],
                        outs=[rs_outs[idx][:, :]],
                    )
                    nc.sync.dma_start(
                        out[idx * 64 : (idx + 1) * 64, :], rs_outs[idx][:, :]
                    )

                for c in range(NS):
                    prev = oproj_list(c - 1) if c > 0 else []
                    nsk = QH * c + QH  # causal: sk tiles for this chunk
                    total_steps = QH * nsk
                    oi = 0
                    si = 0
                    for h in range(QH):
                        esum = esp.tile([P, SC], F16, name="esum", tag="esum")
                        pv = ps_pv.tile([P, SC], F32, name="pv", tag="pv")
                        qrhs = qk[(h, c)][:]
                        for t in range(nsk):
                            kc, kj = divmod(t, NS)
                            ktile = qk[(QH, kc)][:, kj * P : (kj + 1) * P]
                            # diagonal tiles: columns below j*P are fully
                            # masked; compute only the live region
                            lo = (t - QH * c) * P if t >= QH * c else 0
                            sc_ps = ps_sc.tile([P, SC], F32, name="sc_ps", tag="sc")
                            nc.tensor.matmul(
                                sc_ps[:, lo:SC], ktile, qrhs[:, lo:SC],
                                start=True, stop=True,
                            )
                            e = ep.tile([P, SC], F16, name="e", tag="e")
                            nc.scalar.activation(
                                e[:, lo:SC], sc_ps[:, lo:SC], AF.Exp, scale=ISQRT_D
                            )
                            if t >= QH * c:
                                # triangular mask on the diagonal P-block
                                nc.vector.tensor_mul(
                                    e[:, lo : lo + P],
                                    e[:, lo : lo + P],
                                    mask_t[:, 3 * P : 4 * P],
                                )
                            if t == 0:
                                nc.vector.tensor_copy(esum[:], e[:])
                            else:
                                nc.vector.tensor_add(
                                    esum[:, lo:SC], esum[:, lo:SC], e[:, lo:SC]
                                )
                            nc.tensor.matmul(
                                pv[:, lo:SC], vnat[t][:], e[:, lo:SC],
                                start=(t == 0), stop=(t == nsk - 1),
                            )
                            si += 1
                            # interleave the previous chunk's o_proj: start
                            # a few steps in (its attnT normalization chain
                            # is still completing), finish early so its
                            # ReduceScatter fires early
                            start_si = 6
                            end_si = max(start_si + 1, total_steps // 3)
                            frac = (si - start_si) / (end_si - start_si)
                            want = int(len(prev) * min(max(frac, 0.0), 1.0))
                            while prev and oi < want:
                                cc, jj, nn = prev[oi]
                                emit_oproj_group(cc, jj, nn, oi % 2)
                                oi += 1
                        sm = ps_sm.tile([1, SC], F32, name="sm", tag="sm")
                        nc.tensor.matmul(
                            sm[:], ones_t[:], esum[:], start=True, stop=True
                        )
                        # iterative-divide reciprocal is ~6 cyc/elem; the
                        # approx variant (~51 ULP) is 5x faster and far
                        # inside the 2e-2 budget. Reciprocate the [1,512]
                        # row, then broadcast the result.
                        smh = sp.tile([1, SC], F32, name="smh", tag="smh")
                        rcp = sp.tile([1, SC], F32, name="rcp", tag="rcp")
                        bc = sp.tile([P, SC], F32, name="bc", tag="bc")
                        nc.scalar.copy(smh[:], sm[:])
                        nc.vector.reciprocal_approx_fast(rcp[:], smh[:])
                        nc.gpsimd.partition_broadcast(bc[:], rcp[:])
                        nc.vector.tensor_mul(attnT[(h, c)][:], pv[:], bc[:])
                    while oi < len(prev):
                        cc, jj, nn = prev[oi]
                        emit_oproj_group(cc, jj, nn, oi % 2)
                        oi += 1
                    if c > 0:
                        emit_rs(c - 1)

                for idx, (cc, jj, nn) in enumerate(oproj_list(NS - 1)):
                    emit_oproj_group(cc, jj, nn, idx % 2)
                emit_rs(NS - 1)

    nc.compile()
    return nc


def _get_nc():
    if "nc" not in _CACHE:
        _CACHE["nc"] = _build()
    return _CACHE["nc"]


def _host_inputs(positions, hidden_states, Wqkv, Wo):
    """Shard + relayout the full inputs for the 8 cores (fp16 device side)."""
    pos = np.asarray(positions).reshape(-1).astype(np.float64)  # [S]
    hs = np.asarray(hidden_states, dtype=np.float32).reshape(S, HID)
    Wqkv = np.asarray(Wqkv, dtype=np.float32)
    Wo = np.asarray(Wo, dtype=np.float32)

    hT = np.ascontiguousarray(hs.T).astype(np.float16)  # [HID, S]

    half = D // 2
    inv_freq = 1.0 / (THETA ** (np.arange(half, dtype=np.float64) / half))
    ang = pos[None, :] * inv_freq[:, None]  # [64, S]
    cos = np.cos(ang)
    sin = np.sin(ang)
    cos2 = np.ascontiguousarray(np.concatenate([cos, cos], axis=0)).astype(
        np.float16
    )
    sinn2 = np.ascontiguousarray(np.concatenate([-sin, sin], axis=0)).astype(
        np.float16
    )

    # causal mask, [sk, sq] orientation: [zeros(128x384) | upper-tri(128x128)].
    maskd = np.concatenate(
        [np.zeros((P, 3 * P), dtype=np.float16),
         np.triu(np.ones((P, P), dtype=np.float16))], axis=1)
    ident = np.eye(P, dtype=np.float16)
    onesd = np.ones((P, 1), dtype=np.float16)

    qb = Wqkv[:, : H * D]
    kb = Wqkv[:, H * D : H * D + KVH * D]
    vb = Wqkv[:, H * D + KVH * D :]

    in_maps = []
    for c in range(NCORES):
        wq_c = np.concatenate(
            [
                qb[:, c * QH * D : (c + 1) * QH * D],
                kb[:, c * D : (c + 1) * D],
                vb[:, c * D : (c + 1) * D],
            ],
            axis=1,
        ).astype(np.float16)
        wo_c = Wo[c * QH * D : (c + 1) * QH * D, :].astype(np.float16)
        in_maps.append(
            {
                "hT": hT,
                "wqkv": np.ascontiguousarray(wq_c),
                "wo": np.ascontiguousarray(wo_c),
                "cos2": cos2,
                "sinn2": sinn2,
                "maskd": maskd,
                "ident": ident,
                "onesd": onesd,
            }
        )
    return in_maps


def _assemble(results):
    full = np.empty((S, HID), dtype=np.float32)
    for c in range(NCORES):
        oc = np.asarray(results[c]["out"], dtype=np.float32)  # [256, HID]
        for j in range(NS):
            full[SC * j + 64 * c : SC * j + 64 * (c + 1), :] = oc[
                64 * j : 64 * (j + 1), :
            ]
    return full.reshape(1, S, HID)


def kernel(positions, hidden_states, Wqkv, Wo):
    from concourse.bass_utils import run_bass_kernel_spmd

    nc = _get_nc()
    in_maps = _host_inputs(positions, hidden_states, Wqkv, Wo)
    res = run_bass_kernel_spmd(nc, in_maps, core_ids=list(range(NCORES)))
    return _assemble(res.results)


def kernel_timed(positions, hidden_states, Wqkv, Wo, tmpdir="/tmp/ntff_trace"):
    """Like kernel() but with NTFF profiling; returns (output, exec_time_ns)."""
    import os
    import shutil

    from concourse.bass_utils import run_bass_kernel_spmd

    shutil.rmtree(tmpdir, ignore_errors=True)
    os.makedirs(tmpdir, exist_ok=True)
    nc = _get_nc()
    in_maps = _host_inputs(positions, hidden_states, Wqkv, Wo)
    res = run_bass_kernel_spmd(
        nc, in_maps, core_ids=list(range(NCORES)), trace=True, tmpdir=tmpdir
    )
    return _assemble(res.results), res.exec_time_ns
